# revision 20
# baseline (speedup 1.0000x reference)
"""Tensor-parallel multi-head attention kernel for 8 Trainium2 NeuronCores.

Sharding: tensor-parallel over heads. Each core owns 2 of the 16 heads
(a 128-dim slice of the projections). Wq/Wk/Wv are sharded column-wise
(output dim), Wo row-wise (input dim). Each core computes a full-shape
partial of the output projection; the host sums the 8 partials (the
"all-reduce") and transposes back. No device collectives are needed.

All device matmuls run in bf16 with f32 PSUM accumulation.
"""

import sys

if "/opt/trn_rl_repo" not in sys.path:
    sys.path.insert(0, "/opt/trn_rl_repo")

import numpy as np
import ml_dtypes

BF16 = ml_dtypes.bfloat16

B, T, C = 2, 2048, 1024
H, HD = 16, 64
BT = B * T            # 4096 tokens total
NCORES = 8
DPC = C // NCORES     # 128 projection dims per core (2 heads x 64)
NKT = T // 128        # 16 k-tiles of 128 tokens per batch
NCI = C // 128        # 8 contraction tiles for the projections
SCALE = 1.0 / 8.0     # 1/sqrt(HD)
EPS = float(np.finfo(np.float32).eps)
JQ = 512              # query-chunk width in the attention loop
NJQ = T // JQ         # 4 query chunks per batch
GR = 1024             # projection granule (tokens)
NEG = -30000.0        # additive mask bias for padded keys

_CACHE = {}


def _build_bass():
    import concourse.bass as bass
    from concourse import bacc, mybir, tile
    from concourse.masks import make_identity
    from contextlib import ExitStack

    dt = mybir.dt
    AF = mybir.ActivationFunctionType
    ts = bass.ts

    nc = bacc.Bacc("TRN2", target_bir_lowering=False, debug=False)

    import os

    debug = bool(int(os.environ.get("BASS_ATTN_DEBUG", "0")))

    xt_ext = nc.dram_tensor("xt", [128, NCI * BT], dt.bfloat16, kind="ExternalInput")
    wq_ext = nc.dram_tensor("wq", [128, NCI * DPC], dt.bfloat16, kind="ExternalInput")
    wk_ext = nc.dram_tensor("wk", [128, NCI * DPC], dt.bfloat16, kind="ExternalInput")
    wv_ext = nc.dram_tensor("wv", [128, NCI * DPC], dt.bfloat16, kind="ExternalInput")
    wo_ext = nc.dram_tensor("wo", [DPC, C], dt.bfloat16, kind="ExternalInput")
    bias_ext = nc.dram_tensor("bias", [128, B * NKT], dt.float32, kind="ExternalInput")
    out_ext = nc.dram_tensor("out", [128, NCI * BT], dt.bfloat16, kind="ExternalOutput")

    with ExitStack() as ctx:
        tc = ctx.enter_context(tile.TileContext(nc))
        singles = ctx.enter_context(tc.tile_pool(name="singles", bufs=1))
        work = ctx.enter_context(tc.tile_pool(name="work", bufs=2))
        se_pool = ctx.enter_context(tc.tile_pool(name="se", bufs=3))
        rbp = ctx.enter_context(tc.tile_pool(name="rb", bufs=2))
        rb2 = ctx.enter_context(tc.tile_pool(name="rb2", bufs=2))
        pp = ctx.enter_context(tc.tile_pool(name="pp", bufs=2, space="PSUM"))
        pps = ctx.enter_context(tc.tile_pool(name="pps", bufs=4, space="PSUM"))

        # ---- persistent SBUF state ----
        xt_sb = singles.tile([128, NCI * BT], dt.bfloat16)     # xT, ci-major
        wq_sb = singles.tile([128, NCI * DPC], dt.bfloat16)
        wk_sb = singles.tile([128, NCI * DPC], dt.bfloat16)
        wv_sb = singles.tile([128, NCI * DPC], dt.bfloat16)
        wo_sb = singles.tile([128, C], dt.bfloat16)
        bias_sb = singles.tile([128, B * NKT], dt.float32)
        qn_sb = singles.tile([128, BT], dt.bfloat16)           # rms-normed qT
        kn_sb = singles.tile([128, BT], dt.bfloat16)           # rms-normed kT
        # v tiles per (head, batch, k-tile): [v | ones] -> M=65 PV matmuls
        vext = singles.tile([128, 2, B, NKT, HD + 1], dt.bfloat16)
        yn_sb = singles.tile([128, BT], dt.bfloat16)           # normalized yT
        onesA = singles.tile([128, 1], dt.bfloat16)   # 1.0 on partitions 0:64
        onesB = singles.tile([128, 1], dt.bfloat16)   # 1.0 on partitions 64:128
        onesk = singles.tile([128, 1], dt.bfloat16)   # 1.0 everywhere
        ident = singles.tile([128, 128], dt.bfloat16)
        zeros_sb = singles.tile([128, 128], dt.bfloat16)
        eps_sb = singles.tile([128, 1], dt.float32)

        nc.sync.dma_start(out=xt_sb[:], in_=xt_ext.ap())
        nc.sync.dma_start(out=wq_sb[:], in_=wq_ext.ap())
        nc.sync.dma_start(out=wk_sb[:], in_=wk_ext.ap())
        nc.sync.dma_start(out=wv_sb[:], in_=wv_ext.ap())
        nc.sync.dma_start(out=wo_sb[:], in_=wo_ext.ap())
        nc.sync.dma_start(out=bias_sb[:], in_=bias_ext.ap())

        nc.gpsimd.memset(eps_sb[:], EPS)
        nc.gpsimd.memset(onesA[:], 0.0)
        nc.gpsimd.memset(onesA[0:64, :], 1.0)
        nc.gpsimd.memset(onesB[:], 0.0)
        nc.gpsimd.memset(onesB[64:128, :], 1.0)
        nc.gpsimd.memset(onesk[:], 1.0)
        nc.gpsimd.memset(zeros_sb[:], 0.0)
        nc.gpsimd.memset(vext[:, :, :, :, HD : HD + 1], 1.0)
        make_identity(nc, ident[:])

        def proj_psum(w_sb, b, g):
            """Project one granule of GR tokens -> psum [128, GR] (f32)."""
            ps = pp.tile([128, GR], dt.float32, tag="ps")
            t0 = b * T + g * GR
            for ci in range(NCI):
                for ch in range(GR // 512):
                    nc.tensor.matmul(
                        ps[:, ts(ch, 512)],
                        lhsT=w_sb[:, ts(ci, DPC)],
                        rhs=xt_sb[:, ci * BT + t0 + ch * 512 : ci * BT + t0 + (ch + 1) * 512],
                        start=(ci == 0),
                        stop=(ci == NCI - 1),
                    )
            return ps

        def rms_granule(w_sb, dst_sb, b, g):
            """Granule of q or k: project, rms-normalize per head, store bf16."""
            ps = proj_psum(w_sb, b, g)
            t0 = b * T + g * GR
            q2 = work.tile([128, GR], dt.bfloat16, tag="sb1024")
            nc.scalar.activation(out=q2[:], in_=ps[:], func=AF.Square)
            rms = rbp.tile([1, 2 * GR], dt.float32, tag="rms")
            for hh, ones_h in ((0, onesA), (1, onesB)):
                for ch in range(GR // 512):
                    ssq = pps.tile([1, 512], dt.float32, tag="sm")
                    nc.tensor.matmul(
                        ssq[:],
                        lhsT=ones_h[:],
                        rhs=q2[:, ts(ch, 512)],
                        start=True,
                        stop=True,
                    )
                    nc.scalar.activation(
                        out=rms[0:1, hh * GR + ch * 512 : hh * GR + (ch + 1) * 512],
                        in_=ssq[:],
                        func=AF.Sqrt,
                        bias=eps_sb[0:1, :],
                        scale=1.0 / HD,
                    )
            rinv = rms
            nc.vector.reciprocal(out=rinv[:], in_=rms[:])
            rbc = rbp.tile([128, GR], dt.float32, tag="rbc")
            rbB0 = rbp.tile([64, GR], dt.float32, tag="rbB")
            nc.gpsimd.partition_broadcast(rbc[0:64, :], rinv[0:1, 0:GR])
            nc.gpsimd.partition_broadcast(rbB0[:], rinv[0:1, GR : 2 * GR])
            nc.sync.dma_start(out=rbc[64:128, :], in_=rbB0[:])
            nc.vector.tensor_mul(dst_sb[:, t0 : t0 + GR], ps[:], rbc[:])

        def v_granule(b, g):
            """Granule of v: project vT, transpose 128-blocks into vext."""
            ps = proj_psum(wv_sb, b, g)
            vt = work.tile([128, GR], dt.bfloat16, tag="sb1024")
            nc.vector.tensor_copy(vt[:], ps[:])
            for j in range(GR // 128):
                pst = pps.tile([128, 128], dt.bfloat16, tag="sm")
                nc.tensor.transpose(pst[:], vt[:, ts(j, 128)], ident[:])
                kt = g * (GR // 128) + j
                nc.vector.tensor_copy(vext[:, 0, b, kt, 0:HD], pst[:, 0:HD])
                nc.vector.tensor_copy(vext[:, 1, b, kt, 0:HD], pst[:, HD : 2 * HD])

        # ---- phase 1: projections + rms norm ----
        for b in range(B):
            for g in range(T // GR):
                rms_granule(wq_sb, qn_sb, b, g)
                rms_granule(wk_sb, kn_sb, b, g)
                v_granule(b, g)

        # ---- phase 2: attention, phase 3: output projection ----
        def attention_batch(b):
            for jq in range(NJQ):
                q0 = b * T + jq * JQ
                yT = pps.tile([128, JQ], dt.float32, tag="sm")
                sA = pps.tile([1, JQ], dt.float32, tag="sm")
                sB = pps.tile([1, JQ], dt.float32, tag="sm")
                # open one accumulation group covering the whole yT bank
                nc.tensor.matmul(
                    yT[:],
                    lhsT=zeros_sb[:],
                    rhs=xt_sb[:, 0:JQ],
                    start=True,
                    stop=False,
                )
                for kt in range(NKT):
                    k0 = b * T + kt * 128
                    ps_s = pp.tile([128, 2 * JQ], dt.float32, tag="ps")
                    nc.tensor.matmul(
                        ps_s[:, 0:JQ],
                        lhsT=kn_sb[0:64, k0 : k0 + 128],
                        rhs=qn_sb[0:64, q0 : q0 + JQ],
                        start=True,
                        stop=True,
                    )
                    nc.tensor.matmul(
                        ps_s[:, JQ : 2 * JQ],
                        lhsT=kn_sb[64:128, k0 : k0 + 128],
                        rhs=qn_sb[64:128, q0 : q0 + JQ],
                        start=True,
                        stop=True,
                    )
                    se = se_pool.tile([128, 2 * JQ], dt.bfloat16)
                    nc.scalar.activation(
                        out=se[:],
                        in_=ps_s[:],
                        func=AF.Exp,
                        bias=bias_sb[:, b * NKT + kt : b * NKT + kt + 1],
                        scale=SCALE,
                    )
                    nc.tensor.matmul(
                        yT[0:64, :],
                        lhsT=vext[:, 0, b, kt, 0:HD],
                        rhs=se[:, 0:JQ],
                        start=False,
                        stop=False,
                    )
                    nc.tensor.matmul(
                        yT[64:128, :],
                        lhsT=vext[:, 1, b, kt, 0:HD],
                        rhs=se[:, JQ : 2 * JQ],
                        start=False,
                        stop=False,
                    )
                    nc.tensor.matmul(
                        sA[:],
                        lhsT=onesk[:],
                        rhs=se[:, 0:JQ],
                        start=(kt == 0),
                        stop=(kt == NKT - 1),
                    )
                    nc.tensor.matmul(
                        sB[:],
                        lhsT=onesk[:],
                        rhs=se[:, JQ : 2 * JQ],
                        start=(kt == 0),
                        stop=(kt == NKT - 1),
                    )
                # close the yT accumulation group (adds zero)
                nc.tensor.matmul(
                    yT[:],
                    lhsT=zeros_sb[:],
                    rhs=xt_sb[:, 0:JQ],
                    start=False,
                    stop=True,
                )
                # normalize: y / sum(exp); yn rows = [A dims | B dims]
                rs = rb2.tile([1, 2 * JQ], dt.float32, tag="rs")
                nc.vector.reciprocal(out=rs[0:1, 0:JQ], in_=sA[:])
                nc.vector.reciprocal(out=rs[0:1, JQ : 2 * JQ], in_=sB[:])
                rbf = rb2.tile([128, JQ], dt.float32, tag="rbf")
                rbB0 = rb2.tile([64, JQ], dt.float32, tag="rb")
                nc.gpsimd.partition_broadcast(rbf[0:64, :], rs[0:1, 0:JQ])
                nc.gpsimd.partition_broadcast(rbB0[:], rs[0:1, JQ : 2 * JQ])
                nc.sync.dma_start(out=rbf[64:128, :], in_=rbB0[:])
                nc.vector.tensor_mul(yn_sb[:, q0 : q0 + JQ], yT[:], rbf[:])

        def outproj_batch(b):
            for dtile in range(NCI):
                ob = work.tile([128, T], dt.bfloat16, tag="ob")
                for ch in range(T // 512):
                    ps_o = pps.tile([128, 512], dt.float32, tag="sm")
                    nc.tensor.matmul(
                        ps_o[:],
                        lhsT=wo_sb[:, ts(dtile, 128)],
                        rhs=yn_sb[:, b * T + ch * 512 : b * T + (ch + 1) * 512],
                        start=True,
                        stop=True,
                    )
                    nc.vector.tensor_copy(ob[:, ts(ch, 512)], ps_o[:])
                nc.sync.dma_start(
                    out=out_ext.ap()[:, dtile * BT + b * T : dtile * BT + (b + 1) * T],
                    in_=ob[:],
                )

        for b in range(B):
            attention_batch(b)
            outproj_batch(b)

        if debug:
            dbg_specs = [
                ("dbg_qn", qn_sb, [128, BT], dt.bfloat16),
                ("dbg_kn", kn_sb, [128, BT], dt.bfloat16),
                ("dbg_yn", yn_sb, [128, BT], dt.bfloat16),
                ("dbg_vext", vext, [128, 2 * B * NKT * (HD + 1)], dt.bfloat16),
            ]
            for name, src, shape, dty in dbg_specs:
                ext = nc.dram_tensor(name, shape, dty, kind="ExternalOutput")
                flat = src[:]
                if len(flat.shape) > 2:
                    flat = flat.rearrange("p a b c d -> p (a b c d)")
                nc.sync.dma_start(out=ext.ap(), in_=flat)

    nc._dbg = {
        "qn": qn_sb.tensor.name,
        "kn": kn_sb.tensor.name,
        "vext": vext.tensor.name,
        "yn": yn_sb.tensor.name,
        "xt": xt_sb.tensor.name,
        "wq": wq_sb.tensor.name,
    }
    nc.compile()
    return nc


def _get_nc():
    if "nc" not in _CACHE:
        _CACHE["nc"] = _build_bass()
    return _CACHE["nc"]


def _tile_major(a, width):
    """[C, width] -> [128, NCI*width]: c-tile-major columns, partition = c%128."""
    return np.ascontiguousarray(
        a.reshape(NCI, 128, width).transpose(1, 0, 2).reshape(128, NCI * width)
    )


def _prep_in_maps(x, padding_mask, Wq, Wk, Wv, Wo):
    xf = np.ascontiguousarray(np.asarray(x, dtype=np.float32).reshape(BT, C))
    xt = _tile_major(np.ascontiguousarray(xf.T), BT).astype(BF16)
    mb = np.where(
        np.asarray(padding_mask).reshape(BT), np.float32(0.0), np.float32(NEG)
    ).astype(np.float32)
    bias = np.ascontiguousarray(mb.reshape(B * NKT, 128).T)

    in_maps = []
    for i in range(NCORES):
        sl = slice(i * DPC, (i + 1) * DPC)
        in_maps.append(
            {
                "xt": xt,
                "wq": _tile_major(np.ascontiguousarray(Wq[sl, :].T), DPC).astype(BF16),
                "wk": _tile_major(np.ascontiguousarray(Wk[sl, :].T), DPC).astype(BF16),
                "wv": _tile_major(np.ascontiguousarray(Wv[sl, :].T), DPC).astype(BF16),
                "wo": np.ascontiguousarray(Wo[:, sl].T).astype(BF16),
                "bias": bias,
            }
        )
    return in_maps


def _assemble(results):
    total = np.zeros((NCI, 128, BT), dtype=np.float32)
    for r in results:
        total += (
            r["out"].reshape(128, NCI, BT).transpose(1, 0, 2).astype(np.float32)
        )
    return np.ascontiguousarray(total.reshape(C, BT).T).reshape(B, T, C)


def kernel(x, padding_mask, Wq, Wk, Wv, Wo):
    from concourse.bass_utils import run_bass_kernel_spmd

    nc = _get_nc()
    in_maps = _prep_in_maps(x, padding_mask, Wq, Wk, Wv, Wo)
    res = run_bass_kernel_spmd(nc, in_maps, core_ids=list(range(NCORES)))
    return _assemble(res.results)


# revision 22
# speedup vs baseline: 1.2536x; 1.2536x over previous
"""Tensor-parallel multi-head attention kernel for 8 Trainium2 NeuronCores.

Sharding: tensor-parallel over heads. Each core owns 2 of the 16 heads
(a 128-dim slice of the projections). Wq/Wk/Wv are sharded column-wise
(output dim), Wo row-wise (input dim). Each core computes a full-shape
partial of the output projection; the host sums the 8 partials (the
"all-reduce") and transposes back. No device collectives are needed.

All device matmuls run in bf16 with f32 PSUM accumulation.
"""

import sys

if "/opt/trn_rl_repo" not in sys.path:
    sys.path.insert(0, "/opt/trn_rl_repo")

import numpy as np
import ml_dtypes

BF16 = ml_dtypes.bfloat16

B, T, C = 2, 2048, 1024
H, HD = 16, 64
BT = B * T            # 4096 tokens total
NCORES = 8
DPC = C // NCORES     # 128 projection dims per core (2 heads x 64)
NKT = T // 128        # 16 k-tiles of 128 tokens per batch
NCI = C // 128        # 8 contraction tiles for the projections
SCALE = 1.0 / 8.0     # 1/sqrt(HD)
EPS = float(np.finfo(np.float32).eps)
JQ = 512              # query-chunk width in the attention loop
NJQ = T // JQ         # 4 query chunks per batch
GR = 1024             # projection granule (tokens)
NEG = -30000.0        # additive mask bias for padded keys

_CACHE = {}


def _build_bass():
    import concourse.bass as bass
    from concourse import bacc, mybir, tile
    from concourse.masks import make_identity
    from contextlib import ExitStack

    dt = mybir.dt
    AF = mybir.ActivationFunctionType
    ts = bass.ts

    nc = bacc.Bacc("TRN2", target_bir_lowering=False, debug=False)

    import os

    debug = bool(int(os.environ.get("BASS_ATTN_DEBUG", "0")))

    xt_ext = nc.dram_tensor("xt", [128, NCI * BT], dt.bfloat16, kind="ExternalInput")
    wq_ext = nc.dram_tensor("wq", [128, NCI * DPC], dt.bfloat16, kind="ExternalInput")
    wk_ext = nc.dram_tensor("wk", [128, NCI * DPC], dt.bfloat16, kind="ExternalInput")
    wv_ext = nc.dram_tensor("wv", [128, NCI * DPC], dt.bfloat16, kind="ExternalInput")
    wo_ext = nc.dram_tensor("wo", [DPC, C], dt.bfloat16, kind="ExternalInput")
    bias_ext = nc.dram_tensor("bias", [128, B * NKT], dt.float32, kind="ExternalInput")
    out_ext = nc.dram_tensor("out", [128, NCI * BT], dt.bfloat16, kind="ExternalOutput")

    with ExitStack() as ctx:
        tc = ctx.enter_context(tile.TileContext(nc))
        singles = ctx.enter_context(tc.tile_pool(name="singles", bufs=1))
        work = ctx.enter_context(tc.tile_pool(name="work", bufs=2))
        se_pool = ctx.enter_context(tc.tile_pool(name="se", bufs=3))
        rbp = ctx.enter_context(tc.tile_pool(name="rb", bufs=2))
        rb2 = ctx.enter_context(tc.tile_pool(name="rb2", bufs=2))
        pp = ctx.enter_context(tc.tile_pool(name="pp", bufs=2, space="PSUM"))
        pps = ctx.enter_context(tc.tile_pool(name="pps", bufs=4, space="PSUM"))

        # ---- persistent SBUF state ----
        xt_sb = singles.tile([128, NCI * BT], dt.bfloat16)     # xT, ci-major
        wq_sb = singles.tile([128, NCI * DPC], dt.bfloat16)
        wk_sb = singles.tile([128, NCI * DPC], dt.bfloat16)
        wv_sb = singles.tile([128, NCI * DPC], dt.bfloat16)
        wo_sb = singles.tile([128, C], dt.bfloat16)
        bias_sb = singles.tile([128, B * NKT], dt.float32)
        qn_sb = singles.tile([128, BT], dt.bfloat16)           # rms-normed qT
        kn_sb = singles.tile([128, BT], dt.bfloat16)           # rms-normed kT
        # v tiles per (head, batch, k-tile): [v | ones] -> M=65 PV matmuls
        vext = singles.tile([128, 2, B, NKT, HD + 1], dt.bfloat16)
        yn_sb = singles.tile([128, BT], dt.bfloat16)           # normalized yT
        onesA = singles.tile([128, 1], dt.bfloat16)   # 1.0 on partitions 0:64
        onesB = singles.tile([128, 1], dt.bfloat16)   # 1.0 on partitions 64:128
        onesk = singles.tile([128, 1], dt.bfloat16)   # 1.0 everywhere
        ident = singles.tile([128, 128], dt.bfloat16)
        eps_sb = singles.tile([128, 1], dt.float32)

        nc.sync.dma_start(out=xt_sb[:], in_=xt_ext.ap())
        nc.sync.dma_start(out=wq_sb[:], in_=wq_ext.ap())
        nc.sync.dma_start(out=wk_sb[:], in_=wk_ext.ap())
        nc.sync.dma_start(out=wv_sb[:], in_=wv_ext.ap())
        nc.sync.dma_start(out=wo_sb[:], in_=wo_ext.ap())
        nc.sync.dma_start(out=bias_sb[:], in_=bias_ext.ap())

        nc.gpsimd.memset(eps_sb[:], EPS)
        nc.gpsimd.memset(onesA[:], 0.0)
        nc.gpsimd.memset(onesA[0:64, :], 1.0)
        nc.gpsimd.memset(onesB[:], 0.0)
        nc.gpsimd.memset(onesB[64:128, :], 1.0)
        nc.gpsimd.memset(onesk[:], 1.0)
        nc.gpsimd.memset(vext[:, :, :, :, HD : HD + 1], 1.0)
        make_identity(nc, ident[:])

        def proj_psum(w_sb, b, g):
            """Project one granule of GR tokens -> psum [128, GR] (f32)."""
            ps = pp.tile([128, GR], dt.float32, tag="ps")
            t0 = b * T + g * GR
            for ci in range(NCI):
                for ch in range(GR // 512):
                    nc.tensor.matmul(
                        ps[:, ts(ch, 512)],
                        lhsT=w_sb[:, ts(ci, DPC)],
                        rhs=xt_sb[:, ci * BT + t0 + ch * 512 : ci * BT + t0 + (ch + 1) * 512],
                        start=(ci == 0),
                        stop=(ci == NCI - 1),
                    )
            return ps

        def rms_post(ps, dst_sb, b, g):
            """RMS-normalize a projected granule per head, store bf16."""
            t0 = b * T + g * GR
            q2 = work.tile([128, GR], dt.bfloat16, tag="sb1024")
            nc.scalar.activation(out=q2[:], in_=ps[:], func=AF.Square)
            rms = rbp.tile([1, 2 * GR], dt.float32, tag="rms")
            for hh, ones_h in ((0, onesA), (1, onesB)):
                for ch in range(GR // 512):
                    ssq = pps.tile([1, 512], dt.float32, tag="sm")
                    nc.tensor.matmul(
                        ssq[:],
                        lhsT=ones_h[:],
                        rhs=q2[:, ts(ch, 512)],
                        start=True,
                        stop=True,
                    )
                    nc.scalar.activation(
                        out=rms[0:1, hh * GR + ch * 512 : hh * GR + (ch + 1) * 512],
                        in_=ssq[:],
                        func=AF.Sqrt,
                        bias=eps_sb[0:1, :],
                        scale=1.0 / HD,
                    )
            rinv = rms
            nc.vector.reciprocal(out=rinv[:], in_=rms[:])
            rbc = rbp.tile([128, GR], dt.float32, tag="rbc")
            rbB0 = rbp.tile([64, GR], dt.float32, tag="rbB")
            nc.gpsimd.partition_broadcast(rbc[0:64, :], rinv[0:1, 0:GR])
            nc.gpsimd.partition_broadcast(rbB0[:], rinv[0:1, GR : 2 * GR])
            nc.sync.dma_start(out=rbc[64:128, :], in_=rbB0[:])
            nc.vector.tensor_mul(dst_sb[:, t0 : t0 + GR], ps[:], rbc[:])

        def v_post(ps, b, g):
            """Transpose a projected vT granule into vext 128-blocks."""
            vt = work.tile([128, GR], dt.bfloat16, tag="sb1024")
            nc.scalar.copy(vt[:], ps[:])
            for j in range(GR // 128):
                pst = pps.tile([128, 128], dt.bfloat16, tag="sm")
                nc.tensor.transpose(pst[:], vt[:, ts(j, 128)], ident[:])
                kt = g * (GR // 128) + j
                nc.vector.tensor_copy(vext[:, 0, b, kt, 0:HD], pst[:, 0:HD])
                nc.vector.tensor_copy(vext[:, 1, b, kt, 0:HD], pst[:, HD : 2 * HD])

        # ---- phase 1: projections + rms norm (software-pipelined) ----
        specs = []
        for b in range(B):
            for g in range(T // GR):
                specs.append(("q", wq_sb, qn_sb, b, g))
                specs.append(("k", wk_sb, kn_sb, b, g))
                specs.append(("v", wv_sb, None, b, g))
        pending = None
        for kind, w_sb, dst, b, g in specs:
            ps = proj_psum(w_sb, b, g)
            if pending is not None:
                pending()
            if kind == "v":
                pending = (lambda ps=ps, b=b, g=g: v_post(ps, b, g))
            else:
                pending = (lambda ps=ps, dst=dst, b=b, g=g: rms_post(ps, dst, b, g))
        pending()

        # ---- phase 2: attention, phase 3: output projection ----
        def attention_batch(b):
            def qk_tile(kt):
                k0 = b * T + kt * 128
                ps_s = pp.tile([128, 2 * JQ], dt.float32, tag="ps")
                nc.tensor.matmul(
                    ps_s[:, 0:JQ],
                    lhsT=kn_sb[0:64, k0 : k0 + 128],
                    rhs=qn_sb[0:64, q0 : q0 + JQ],
                    start=True,
                    stop=True,
                )
                nc.tensor.matmul(
                    ps_s[:, JQ : 2 * JQ],
                    lhsT=kn_sb[64:128, k0 : k0 + 128],
                    rhs=qn_sb[64:128, q0 : q0 + JQ],
                    start=True,
                    stop=True,
                )
                return ps_s

            for jq in range(NJQ):
                q0 = b * T + jq * JQ
                yA = pps.tile([HD + 1, JQ], dt.float32, tag="sm")
                yB = pps.tile([HD + 1, JQ], dt.float32, tag="sm")
                ps_cur = qk_tile(0)
                for kt in range(NKT):
                    se = se_pool.tile([128, 2 * JQ], dt.bfloat16)
                    nc.scalar.activation(
                        out=se[:],
                        in_=ps_cur[:],
                        func=AF.Exp,
                        bias=bias_sb[:, b * NKT + kt : b * NKT + kt + 1],
                        scale=SCALE,
                    )
                    if kt + 1 < NKT:
                        ps_cur = qk_tile(kt + 1)
                    nc.tensor.matmul(
                        yA[:],
                        lhsT=vext[:, 0, b, kt, :],
                        rhs=se[:, 0:JQ],
                        start=(kt == 0),
                        stop=(kt == NKT - 1),
                    )
                    nc.tensor.matmul(
                        yB[:],
                        lhsT=vext[:, 1, b, kt, :],
                        rhs=se[:, JQ : 2 * JQ],
                        start=(kt == 0),
                        stop=(kt == NKT - 1),
                    )
                # normalize: y / sum(exp); sums sit in row 64 of yA/yB
                rsv = rb2.tile([65, 2 * JQ], dt.float32, tag="rs")
                nc.vector.reciprocal(out=rsv[64:65, 0:JQ], in_=yA[HD : HD + 1, :])
                nc.vector.reciprocal(
                    out=rsv[64:65, JQ : 2 * JQ], in_=yB[HD : HD + 1, :]
                )
                rs0 = rb2.tile([1, 2 * JQ], dt.float32, tag="rs0")
                nc.sync.dma_start(out=rs0[:], in_=rsv[64:65, :])
                rbA = rb2.tile([64, JQ], dt.float32, tag="rb")
                rbB0 = rb2.tile([64, JQ], dt.float32, tag="rb")
                nc.gpsimd.partition_broadcast(rbA[:], rs0[0:1, 0:JQ])
                nc.gpsimd.partition_broadcast(rbB0[:], rs0[0:1, JQ : 2 * JQ])
                nc.vector.tensor_mul(yn_sb[0:64, q0 : q0 + JQ], yA[0:64, :], rbA[:])
                ynB = work.tile([64, JQ], dt.bfloat16, tag="ynB")
                nc.vector.tensor_mul(ynB[:], yB[0:64, :], rbB0[:])
                nc.sync.dma_start(out=yn_sb[64:128, q0 : q0 + JQ], in_=ynB[:])

        def outproj_batch(b):
            for dtile in range(NCI):
                ob = work.tile([128, T], dt.bfloat16, tag="ob")
                for ch in range(T // 512):
                    ps_o = pps.tile([128, 512], dt.float32, tag="sm")
                    nc.tensor.matmul(
                        ps_o[:],
                        lhsT=wo_sb[:, ts(dtile, 128)],
                        rhs=yn_sb[:, b * T + ch * 512 : b * T + (ch + 1) * 512],
                        start=True,
                        stop=True,
                    )
                    nc.vector.tensor_copy(ob[:, ts(ch, 512)], ps_o[:])
                nc.sync.dma_start(
                    out=out_ext.ap()[:, dtile * BT + b * T : dtile * BT + (b + 1) * T],
                    in_=ob[:],
                )

        for b in range(B):
            attention_batch(b)
            outproj_batch(b)

        if debug:
            dbg_specs = [
                ("dbg_qn", qn_sb, [128, BT], dt.bfloat16),
                ("dbg_kn", kn_sb, [128, BT], dt.bfloat16),
                ("dbg_yn", yn_sb, [128, BT], dt.bfloat16),
                ("dbg_vext", vext, [128, 2 * B * NKT * (HD + 1)], dt.bfloat16),
            ]
            for name, src, shape, dty in dbg_specs:
                ext = nc.dram_tensor(name, shape, dty, kind="ExternalOutput")
                flat = src[:]
                if len(flat.shape) > 2:
                    flat = flat.rearrange("p a b c d -> p (a b c d)")
                nc.sync.dma_start(out=ext.ap(), in_=flat)

    nc._dbg = {
        "qn": qn_sb.tensor.name,
        "kn": kn_sb.tensor.name,
        "vext": vext.tensor.name,
        "yn": yn_sb.tensor.name,
        "xt": xt_sb.tensor.name,
        "wq": wq_sb.tensor.name,
    }
    nc.compile()
    return nc


def _get_nc():
    if "nc" not in _CACHE:
        _CACHE["nc"] = _build_bass()
    return _CACHE["nc"]


def _tile_major(a, width):
    """[C, width] -> [128, NCI*width]: c-tile-major columns, partition = c%128."""
    return np.ascontiguousarray(
        a.reshape(NCI, 128, width).transpose(1, 0, 2).reshape(128, NCI * width)
    )


def _prep_in_maps(x, padding_mask, Wq, Wk, Wv, Wo):
    xf = np.ascontiguousarray(np.asarray(x, dtype=np.float32).reshape(BT, C))
    xt = _tile_major(np.ascontiguousarray(xf.T), BT).astype(BF16)
    mb = np.where(
        np.asarray(padding_mask).reshape(BT), np.float32(0.0), np.float32(NEG)
    ).astype(np.float32)
    bias = np.ascontiguousarray(mb.reshape(B * NKT, 128).T)

    in_maps = []
    for i in range(NCORES):
        sl = slice(i * DPC, (i + 1) * DPC)
        in_maps.append(
            {
                "xt": xt,
                "wq": _tile_major(np.ascontiguousarray(Wq[sl, :].T), DPC).astype(BF16),
                "wk": _tile_major(np.ascontiguousarray(Wk[sl, :].T), DPC).astype(BF16),
                "wv": _tile_major(np.ascontiguousarray(Wv[sl, :].T), DPC).astype(BF16),
                "wo": np.ascontiguousarray(Wo[:, sl].T).astype(BF16),
                "bias": bias,
            }
        )
    return in_maps


def _assemble(results):
    total = np.zeros((NCI, 128, BT), dtype=np.float32)
    for r in results:
        total += (
            r["out"].reshape(128, NCI, BT).transpose(1, 0, 2).astype(np.float32)
        )
    return np.ascontiguousarray(total.reshape(C, BT).T).reshape(B, T, C)


def kernel(x, padding_mask, Wq, Wk, Wv, Wo):
    from concourse.bass_utils import run_bass_kernel_spmd

    nc = _get_nc()
    in_maps = _prep_in_maps(x, padding_mask, Wq, Wk, Wv, Wo)
    res = run_bass_kernel_spmd(nc, in_maps, core_ids=list(range(NCORES)))
    return _assemble(res.results)


# revision 25
# speedup vs baseline: 1.5137x; 1.2075x over previous
"""Tensor-parallel multi-head attention kernel for 8 Trainium2 NeuronCores.

Sharding: tensor-parallel over heads. Each core owns 2 of the 16 heads
(a 128-dim slice of the projections). Wq/Wk/Wv are sharded column-wise
(output dim), Wo row-wise (input dim). Each core computes a full-shape
partial of the output projection; the host sums the 8 partials (the
"all-reduce") and transposes back. No device collectives are needed.

All device matmuls run in bf16 with f32 PSUM accumulation.
"""

import sys

if "/opt/trn_rl_repo" not in sys.path:
    sys.path.insert(0, "/opt/trn_rl_repo")

import numpy as np
import ml_dtypes

BF16 = ml_dtypes.bfloat16

B, T, C = 2, 2048, 1024
H, HD = 16, 64
BT = B * T            # 4096 tokens total
NCORES = 8
DPC = C // NCORES     # 128 projection dims per core (2 heads x 64)
NKT = T // 128        # 16 k-tiles of 128 tokens per batch
NCI = C // 128        # 8 contraction tiles for the projections
SCALE = 1.0 / 8.0     # 1/sqrt(HD)
EPS = float(np.finfo(np.float32).eps)
JQ = 512              # query-chunk width in the attention loop
NJQ = T // JQ         # 4 query chunks per batch
GR = 1024             # projection granule (tokens)
NEG = -30000.0        # additive mask bias for padded keys

_CACHE = {}


def _build_bass():
    import concourse.bass as bass
    from concourse import bacc, mybir, tile
    from concourse.masks import make_identity
    from contextlib import ExitStack

    dt = mybir.dt
    AF = mybir.ActivationFunctionType
    ts = bass.ts

    nc = bacc.Bacc("TRN2", target_bir_lowering=False, debug=False)

    import os

    debug = bool(int(os.environ.get("BASS_ATTN_DEBUG", "0")))

    xt_ext = nc.dram_tensor("xt", [128, NCI * BT], dt.bfloat16, kind="ExternalInput")
    wq_ext = nc.dram_tensor("wq", [128, NCI * DPC], dt.bfloat16, kind="ExternalInput")
    wk_ext = nc.dram_tensor("wk", [128, NCI * DPC], dt.bfloat16, kind="ExternalInput")
    wv_ext = nc.dram_tensor("wv", [128, NCI * DPC], dt.bfloat16, kind="ExternalInput")
    wo_ext = nc.dram_tensor("wo", [DPC, C], dt.bfloat16, kind="ExternalInput")
    bias_ext = nc.dram_tensor("bias", [128, B * NKT], dt.float32, kind="ExternalInput")
    out_ext = nc.dram_tensor("out", [128, NCI * BT], dt.bfloat16, kind="ExternalOutput")

    with ExitStack() as ctx:
        tc = ctx.enter_context(tile.TileContext(nc))
        singles = ctx.enter_context(tc.tile_pool(name="singles", bufs=1))
        work = ctx.enter_context(tc.tile_pool(name="work", bufs=2))
        se_pool = ctx.enter_context(tc.tile_pool(name="se", bufs=3))
        rbp = ctx.enter_context(tc.tile_pool(name="rb", bufs=2))
        rb2 = ctx.enter_context(tc.tile_pool(name="rb2", bufs=2))
        pp = ctx.enter_context(tc.tile_pool(name="pp", bufs=2, space="PSUM"))
        pps = ctx.enter_context(tc.tile_pool(name="pps", bufs=4, space="PSUM"))

        # ---- persistent SBUF state ----
        xt_sb = singles.tile([128, NCI * BT], dt.bfloat16)     # xT, ci-major
        wq_sb = singles.tile([128, NCI * DPC], dt.bfloat16)
        wk_sb = singles.tile([128, NCI * DPC], dt.bfloat16)
        wv_sb = singles.tile([128, NCI * DPC], dt.bfloat16)
        wo_sb = singles.tile([128, C], dt.bfloat16)
        bias_sb = singles.tile([128, B * NKT], dt.float32)
        qn_sb = singles.tile([128, BT], dt.bfloat16)           # rms-normed qT
        kn_sb = singles.tile([128, BT], dt.bfloat16)           # rms-normed kT
        # v tiles per (head, batch, k-tile): [v | ones] -> M=65 PV matmuls
        vext = singles.tile([128, 2, B, NKT, HD + 1], dt.bfloat16)
        yn_sb = singles.tile([128, BT], dt.bfloat16)           # normalized yT
        onesA = singles.tile([128, 1], dt.bfloat16)   # 1.0 on partitions 0:64
        onesB = singles.tile([128, 1], dt.bfloat16)   # 1.0 on partitions 64:128
        onesk = singles.tile([128, 1], dt.bfloat16)   # 1.0 everywhere
        ident = singles.tile([128, 128], dt.bfloat16)
        eps_sb = singles.tile([128, 1], dt.float32)
        zero_sb = singles.tile([128, 1], dt.float32)

        nc.sync.dma_start(out=xt_sb[:], in_=xt_ext.ap())
        nc.sync.dma_start(out=wq_sb[:], in_=wq_ext.ap())
        nc.sync.dma_start(out=wk_sb[:], in_=wk_ext.ap())
        nc.sync.dma_start(out=wv_sb[:], in_=wv_ext.ap())
        nc.sync.dma_start(out=wo_sb[:], in_=wo_ext.ap())
        nc.sync.dma_start(out=bias_sb[:], in_=bias_ext.ap())

        nc.gpsimd.memset(eps_sb[:], EPS)
        nc.gpsimd.memset(zero_sb[:], 0.0)
        nc.gpsimd.memset(onesA[:], 0.0)
        nc.gpsimd.memset(onesA[0:64, :], 1.0)
        nc.gpsimd.memset(onesB[:], 0.0)
        nc.gpsimd.memset(onesB[64:128, :], 1.0)
        nc.gpsimd.memset(onesk[:], 1.0)
        nc.gpsimd.memset(vext[:, :, :, :, HD : HD + 1], 1.0)
        make_identity(nc, ident[:])

        def proj_psum(w_sb, b, g):
            """Project one granule of GR tokens -> psum [128, GR] (f32)."""
            ps = pp.tile([128, GR], dt.float32, tag="ps")
            t0 = b * T + g * GR
            for ci in range(NCI):
                for ch in range(GR // 512):
                    nc.tensor.matmul(
                        ps[:, ts(ch, 512)],
                        lhsT=w_sb[:, ts(ci, DPC)],
                        rhs=xt_sb[:, ci * BT + t0 + ch * 512 : ci * BT + t0 + (ch + 1) * 512],
                        start=(ci == 0),
                        stop=(ci == NCI - 1),
                    )
            return ps

        def rms_post(ps, dst_sb, b, g):
            """RMS-normalize a projected granule per head, store bf16."""
            t0 = b * T + g * GR
            q2 = work.tile([128, GR], dt.bfloat16, tag="sb1024")
            nc.scalar.activation(out=q2[:], in_=ps[:], func=AF.Square)
            lnt = rbp.tile([1, 2 * GR], dt.float32, tag="lnt")
            for hh, ones_h in ((0, onesA), (1, onesB)):
                for ch in range(GR // 512):
                    ssq = pps.tile([1, 512], dt.float32, tag="sm")
                    nc.tensor.matmul(
                        ssq[:],
                        lhsT=ones_h[:],
                        rhs=q2[:, ts(ch, 512)],
                        start=True,
                        stop=True,
                    )
                    nc.scalar.activation(
                        out=lnt[0:1, hh * GR + ch * 512 : hh * GR + (ch + 1) * 512],
                        in_=ssq[:],
                        func=AF.Ln,
                        bias=eps_sb[0:1, :],
                        scale=1.0 / HD,
                    )
            rinv = rbp.tile([1, 2 * GR], dt.float32, tag="rms")
            # rinv = exp(-0.5 * ln(ms + eps)) = rsqrt(ms + eps)
            nc.scalar.activation(
                out=rinv[:], in_=lnt[:], func=AF.Exp, bias=zero_sb[0:1, :], scale=-0.5
            )
            rbc = rbp.tile([128, GR], dt.float32, tag="rbc")
            rbB0 = rbp.tile([64, GR], dt.float32, tag="rbB")
            nc.gpsimd.partition_broadcast(rbc[0:64, :], rinv[0:1, 0:GR])
            nc.gpsimd.partition_broadcast(rbB0[:], rinv[0:1, GR : 2 * GR])
            nc.sync.dma_start(out=rbc[64:128, :], in_=rbB0[:])
            nc.vector.tensor_mul(dst_sb[:, t0 : t0 + GR], ps[:], rbc[:])

        def v_post(ps, b, g):
            """Transpose a projected vT granule into vext 128-blocks."""
            vt = work.tile([128, GR], dt.bfloat16, tag="sb1024")
            nc.scalar.copy(vt[:], ps[:])
            for j in range(GR // 128):
                pst = pps.tile([128, 128], dt.bfloat16, tag="sm")
                nc.tensor.transpose(pst[:], vt[:, ts(j, 128)], ident[:])
                kt = g * (GR // 128) + j
                nc.vector.tensor_copy(vext[:, 0, b, kt, 0:HD], pst[:, 0:HD])
                nc.vector.tensor_copy(vext[:, 1, b, kt, 0:HD], pst[:, HD : 2 * HD])

        # ---- phase 1: projections + rms norm (software-pipelined) ----
        specs = []
        for b in range(B):
            for g in range(T // GR):
                specs.append(("q", wq_sb, qn_sb, b, g))
                specs.append(("k", wk_sb, kn_sb, b, g))
                specs.append(("v", wv_sb, None, b, g))
        pending = None
        for kind, w_sb, dst, b, g in specs:
            ps = proj_psum(w_sb, b, g)
            if pending is not None:
                pending()
            if kind == "v":
                pending = (lambda ps=ps, b=b, g=g: v_post(ps, b, g))
            else:
                pending = (lambda ps=ps, dst=dst, b=b, g=g: rms_post(ps, dst, b, g))
        pending()

        # ---- phase 2: attention, phase 3: output projection ----
        def attention_batch(b):
            def qk_tile(kt):
                k0 = b * T + kt * 128
                ps_s = pp.tile([128, 2 * JQ], dt.float32, tag="ps")
                nc.tensor.matmul(
                    ps_s[:, 0:JQ],
                    lhsT=kn_sb[0:64, k0 : k0 + 128],
                    rhs=qn_sb[0:64, q0 : q0 + JQ],
                    start=True,
                    stop=True,
                )
                nc.tensor.matmul(
                    ps_s[:, JQ : 2 * JQ],
                    lhsT=kn_sb[64:128, k0 : k0 + 128],
                    rhs=qn_sb[64:128, q0 : q0 + JQ],
                    start=True,
                    stop=True,
                )
                return ps_s

            for jq in range(NJQ):
                q0 = b * T + jq * JQ
                yA = pps.tile([HD + 1, JQ], dt.float32, tag="sm")
                yB = pps.tile([HD + 1, JQ], dt.float32, tag="sm")
                ps_cur = qk_tile(0)
                for kt in range(NKT):
                    se = se_pool.tile([128, 2 * JQ], dt.bfloat16)
                    nc.scalar.activation(
                        out=se[:],
                        in_=ps_cur[:],
                        func=AF.Exp,
                        bias=bias_sb[:, b * NKT + kt : b * NKT + kt + 1],
                        scale=SCALE,
                    )
                    if kt + 1 < NKT:
                        ps_cur = qk_tile(kt + 1)
                    nc.tensor.matmul(
                        yA[:],
                        lhsT=vext[:, 0, b, kt, :],
                        rhs=se[:, 0:JQ],
                        start=(kt == 0),
                        stop=(kt == NKT - 1),
                    )
                    nc.tensor.matmul(
                        yB[:],
                        lhsT=vext[:, 1, b, kt, :],
                        rhs=se[:, JQ : 2 * JQ],
                        start=(kt == 0),
                        stop=(kt == NKT - 1),
                    )
                # normalize: y / sum(exp); sums sit in row 64 of yA/yB
                rsv = rb2.tile([65, 2 * JQ], dt.float32, tag="rs")
                nc.vector.tensor_copy(rsv[64:65, 0:JQ], yA[HD : HD + 1, :])
                nc.vector.tensor_copy(rsv[64:65, JQ : 2 * JQ], yB[HD : HD + 1, :])
                rs0 = rb2.tile([1, 2 * JQ], dt.float32, tag="rs0")
                nc.sync.dma_start(out=rs0[:], in_=rsv[64:65, :])
                rr = rb2.tile([1, 2 * JQ], dt.float32, tag="rs0")
                nc.vector.reciprocal_approx_fast(out=rr[:], in_=rs0[:])
                rbA = rb2.tile([64, JQ], dt.float32, tag="rb")
                rbB0 = rb2.tile([64, JQ], dt.float32, tag="rb")
                nc.gpsimd.partition_broadcast(rbA[:], rr[0:1, 0:JQ])
                nc.gpsimd.partition_broadcast(rbB0[:], rr[0:1, JQ : 2 * JQ])
                nc.vector.tensor_mul(yn_sb[0:64, q0 : q0 + JQ], yA[0:64, :], rbA[:])
                ynB = work.tile([64, JQ], dt.bfloat16, tag="ynB")
                nc.vector.tensor_mul(ynB[:], yB[0:64, :], rbB0[:])
                nc.sync.dma_start(out=yn_sb[64:128, q0 : q0 + JQ], in_=ynB[:])

        def outproj_batch(b):
            for dtile in range(NCI):
                ob = work.tile([128, T], dt.bfloat16, tag="ob")
                for ch in range(T // 512):
                    ps_o = pps.tile([128, 512], dt.float32, tag="sm")
                    nc.tensor.matmul(
                        ps_o[:],
                        lhsT=wo_sb[:, ts(dtile, 128)],
                        rhs=yn_sb[:, b * T + ch * 512 : b * T + (ch + 1) * 512],
                        start=True,
                        stop=True,
                    )
                    nc.vector.tensor_copy(ob[:, ts(ch, 512)], ps_o[:])
                nc.sync.dma_start(
                    out=out_ext.ap()[:, dtile * BT + b * T : dtile * BT + (b + 1) * T],
                    in_=ob[:],
                )

        for b in range(B):
            attention_batch(b)
            outproj_batch(b)

        if debug:
            dbg_specs = [
                ("dbg_qn", qn_sb, [128, BT], dt.bfloat16),
                ("dbg_kn", kn_sb, [128, BT], dt.bfloat16),
                ("dbg_yn", yn_sb, [128, BT], dt.bfloat16),
                ("dbg_vext", vext, [128, 2 * B * NKT * (HD + 1)], dt.bfloat16),
            ]
            for name, src, shape, dty in dbg_specs:
                ext = nc.dram_tensor(name, shape, dty, kind="ExternalOutput")
                flat = src[:]
                if len(flat.shape) > 2:
                    flat = flat.rearrange("p a b c d -> p (a b c d)")
                nc.sync.dma_start(out=ext.ap(), in_=flat)

    nc._dbg = {
        "qn": qn_sb.tensor.name,
        "kn": kn_sb.tensor.name,
        "vext": vext.tensor.name,
        "yn": yn_sb.tensor.name,
        "xt": xt_sb.tensor.name,
        "wq": wq_sb.tensor.name,
    }
    nc.compile()
    return nc


def _get_nc():
    if "nc" not in _CACHE:
        _CACHE["nc"] = _build_bass()
    return _CACHE["nc"]


def _tile_major(a, width):
    """[C, width] -> [128, NCI*width]: c-tile-major columns, partition = c%128."""
    return np.ascontiguousarray(
        a.reshape(NCI, 128, width).transpose(1, 0, 2).reshape(128, NCI * width)
    )


def _prep_in_maps(x, padding_mask, Wq, Wk, Wv, Wo):
    xf = np.ascontiguousarray(np.asarray(x, dtype=np.float32).reshape(BT, C))
    xt = _tile_major(np.ascontiguousarray(xf.T), BT).astype(BF16)
    mb = np.where(
        np.asarray(padding_mask).reshape(BT), np.float32(0.0), np.float32(NEG)
    ).astype(np.float32)
    bias = np.ascontiguousarray(mb.reshape(B * NKT, 128).T)

    in_maps = []
    for i in range(NCORES):
        sl = slice(i * DPC, (i + 1) * DPC)
        in_maps.append(
            {
                "xt": xt,
                "wq": _tile_major(np.ascontiguousarray(Wq[sl, :].T), DPC).astype(BF16),
                "wk": _tile_major(np.ascontiguousarray(Wk[sl, :].T), DPC).astype(BF16),
                "wv": _tile_major(np.ascontiguousarray(Wv[sl, :].T), DPC).astype(BF16),
                "wo": np.ascontiguousarray(Wo[:, sl].T).astype(BF16),
                "bias": bias,
            }
        )
    return in_maps


def _assemble(results):
    total = np.zeros((NCI, 128, BT), dtype=np.float32)
    for r in results:
        total += (
            r["out"].reshape(128, NCI, BT).transpose(1, 0, 2).astype(np.float32)
        )
    return np.ascontiguousarray(total.reshape(C, BT).T).reshape(B, T, C)


def kernel(x, padding_mask, Wq, Wk, Wv, Wo):
    from concourse.bass_utils import run_bass_kernel_spmd

    nc = _get_nc()
    in_maps = _prep_in_maps(x, padding_mask, Wq, Wk, Wv, Wo)
    res = run_bass_kernel_spmd(nc, in_maps, core_ids=list(range(NCORES)))
    return _assemble(res.results)


# revision 26
# speedup vs baseline: 1.6602x; 1.0968x over previous
"""Tensor-parallel multi-head attention kernel for 8 Trainium2 NeuronCores.

Sharding: tensor-parallel over heads. Each core owns 2 of the 16 heads
(a 128-dim slice of the projections). Wq/Wk/Wv are sharded column-wise
(output dim), Wo row-wise (input dim). Each core computes a full-shape
partial of the output projection; the host sums the 8 partials (the
"all-reduce") and transposes back. No device collectives are needed.

All device matmuls run in bf16 with f32 PSUM accumulation.
"""

import sys

if "/opt/trn_rl_repo" not in sys.path:
    sys.path.insert(0, "/opt/trn_rl_repo")

import numpy as np
import ml_dtypes

BF16 = ml_dtypes.bfloat16

B, T, C = 2, 2048, 1024
H, HD = 16, 64
BT = B * T            # 4096 tokens total
NCORES = 8
DPC = C // NCORES     # 128 projection dims per core (2 heads x 64)
NKT = T // 128        # 16 k-tiles of 128 tokens per batch
NCI = C // 128        # 8 contraction tiles for the projections
SCALE = 1.0 / 8.0     # 1/sqrt(HD)
EPS = float(np.finfo(np.float32).eps)
JQ = 512              # query-chunk width in the attention loop
NJQ = T // JQ         # 4 query chunks per batch
GR = 1024             # projection granule (tokens)
NEG = -30000.0        # additive mask bias for padded keys

_CACHE = {}


def _build_bass():
    import concourse.bass as bass
    from concourse import bacc, mybir, tile
    from concourse.masks import make_identity
    from contextlib import ExitStack

    dt = mybir.dt
    AF = mybir.ActivationFunctionType
    ts = bass.ts

    nc = bacc.Bacc("TRN2", target_bir_lowering=False, debug=False)

    import os

    debug = bool(int(os.environ.get("BASS_ATTN_DEBUG", "0")))

    xt_ext = nc.dram_tensor("xt", [128, NCI * BT], dt.bfloat16, kind="ExternalInput")
    wq_ext = nc.dram_tensor("wq", [128, NCI * DPC], dt.bfloat16, kind="ExternalInput")
    wk_ext = nc.dram_tensor("wk", [128, NCI * DPC], dt.bfloat16, kind="ExternalInput")
    wv_ext = nc.dram_tensor("wv", [128, NCI * DPC], dt.bfloat16, kind="ExternalInput")
    wo_ext = nc.dram_tensor("wo", [DPC, C], dt.bfloat16, kind="ExternalInput")
    bias_ext = nc.dram_tensor("bias", [128, B * NKT], dt.float32, kind="ExternalInput")
    out_ext = nc.dram_tensor("out", [128, NCI * BT], dt.bfloat16, kind="ExternalOutput")

    with ExitStack() as ctx:
        tc = ctx.enter_context(tile.TileContext(nc))
        singles = ctx.enter_context(tc.tile_pool(name="singles", bufs=1))
        work = ctx.enter_context(tc.tile_pool(name="work", bufs=2))
        se_pool = ctx.enter_context(tc.tile_pool(name="se", bufs=3))
        rbp = ctx.enter_context(tc.tile_pool(name="rb", bufs=2))
        rb2 = ctx.enter_context(tc.tile_pool(name="rb2", bufs=2))
        pp = ctx.enter_context(tc.tile_pool(name="pp", bufs=2, space="PSUM"))
        pps = ctx.enter_context(tc.tile_pool(name="pps", bufs=4, space="PSUM"))

        # ---- persistent SBUF state ----
        xt_sb = singles.tile([128, NCI * BT], dt.bfloat16)     # xT, ci-major
        wq_sb = singles.tile([128, NCI * DPC], dt.bfloat16)
        wk_sb = singles.tile([128, NCI * DPC], dt.bfloat16)
        wv_sb = singles.tile([128, NCI * DPC], dt.bfloat16)
        wo_sb = singles.tile([128, C], dt.bfloat16)
        bias_sb = singles.tile([128, B * NKT], dt.float32)
        qn_sb = singles.tile([128, BT], dt.bfloat16)           # rms-normed qT
        kn_sb = singles.tile([128, BT], dt.bfloat16)           # rms-normed kT
        # v tiles per (head, batch, k-tile): [v | ones] -> M=65 PV matmuls
        vext = singles.tile([128, 2, B, NKT, HD + 1], dt.bfloat16)
        yn_sb = singles.tile([128, BT], dt.bfloat16)           # normalized yT
        onesA = singles.tile([128, 1], dt.bfloat16)   # 1.0 on partitions 0:64
        onesB = singles.tile([128, 1], dt.bfloat16)   # 1.0 on partitions 64:128
        onesk = singles.tile([128, 1], dt.bfloat16)   # 1.0 everywhere
        ident = singles.tile([128, 128], dt.bfloat16)
        eps_sb = singles.tile([128, 1], dt.float32)
        zero_sb = singles.tile([128, 1], dt.float32)

        nc.sync.dma_start(out=xt_sb[:], in_=xt_ext.ap())
        nc.sync.dma_start(out=wq_sb[:], in_=wq_ext.ap())
        nc.sync.dma_start(out=wk_sb[:], in_=wk_ext.ap())
        nc.sync.dma_start(out=wv_sb[:], in_=wv_ext.ap())
        nc.sync.dma_start(out=wo_sb[:], in_=wo_ext.ap())
        nc.sync.dma_start(out=bias_sb[:], in_=bias_ext.ap())

        nc.gpsimd.memset(eps_sb[:], EPS)
        nc.gpsimd.memset(zero_sb[:], 0.0)
        nc.gpsimd.memset(onesA[:], 0.0)
        nc.gpsimd.memset(onesA[0:64, :], 1.0)
        nc.gpsimd.memset(onesB[:], 0.0)
        nc.gpsimd.memset(onesB[64:128, :], 1.0)
        nc.gpsimd.memset(onesk[:], 1.0)
        nc.gpsimd.memset(vext[:, :, :, :, HD : HD + 1], 1.0)
        make_identity(nc, ident[:])

        def proj_psum(w_sb, b, g):
            """Project one granule of GR tokens -> psum [128, GR] (f32)."""
            ps = pp.tile([128, GR], dt.float32, tag="ps")
            t0 = b * T + g * GR
            for ci in range(NCI):
                for ch in range(GR // 512):
                    nc.tensor.matmul(
                        ps[:, ts(ch, 512)],
                        lhsT=w_sb[:, ts(ci, DPC)],
                        rhs=xt_sb[:, ci * BT + t0 + ch * 512 : ci * BT + t0 + (ch + 1) * 512],
                        start=(ci == 0),
                        stop=(ci == NCI - 1),
                    )
            return ps

        def rms_post(ps, dst_sb, b, g):
            """RMS-normalize a projected granule per head, store bf16."""
            t0 = b * T + g * GR
            q2 = work.tile([128, GR], dt.bfloat16, tag="sb1024")
            nc.scalar.activation(out=q2[:], in_=ps[:], func=AF.Square)
            lnt = rbp.tile([1, 2 * GR], dt.float32, tag="lnt")
            for hh, ones_h in ((0, onesA), (1, onesB)):
                for ch in range(GR // 512):
                    ssq = pps.tile([1, 512], dt.float32, tag="sm")
                    nc.tensor.matmul(
                        ssq[:],
                        lhsT=ones_h[:],
                        rhs=q2[:, ts(ch, 512)],
                        start=True,
                        stop=True,
                    )
                    nc.scalar.activation(
                        out=lnt[0:1, hh * GR + ch * 512 : hh * GR + (ch + 1) * 512],
                        in_=ssq[:],
                        func=AF.Ln,
                        bias=eps_sb[0:1, :],
                        scale=1.0 / HD,
                    )
            rinv = rbp.tile([1, 2 * GR], dt.float32, tag="rms")
            # rinv = exp(-0.5 * ln(ms + eps)) = rsqrt(ms + eps)
            nc.scalar.activation(
                out=rinv[:], in_=lnt[:], func=AF.Exp, bias=zero_sb[0:1, :], scale=-0.5
            )
            rbc = rbp.tile([128, GR], dt.float32, tag="rbc")
            rbB0 = rbp.tile([64, GR], dt.float32, tag="rbB")
            nc.gpsimd.partition_broadcast(rbc[0:64, :], rinv[0:1, 0:GR])
            nc.gpsimd.partition_broadcast(rbB0[:], rinv[0:1, GR : 2 * GR])
            nc.sync.dma_start(out=rbc[64:128, :], in_=rbB0[:])
            nc.vector.tensor_mul(dst_sb[:, t0 : t0 + GR], ps[:], rbc[:])

        def v_post(ps, b, g):
            """Transpose a projected vT granule into vext 128-blocks."""
            vt = work.tile([128, GR], dt.bfloat16, tag="sb1024")
            nc.scalar.copy(vt[:], ps[:])
            for j in range(GR // 128):
                pst = pps.tile([128, 128], dt.bfloat16, tag="sm")
                nc.tensor.transpose(pst[:], vt[:, ts(j, 128)], ident[:])
                kt = g * (GR // 128) + j
                nc.vector.tensor_copy(vext[:, 0, b, kt, 0:HD], pst[:, 0:HD])
                nc.vector.tensor_copy(vext[:, 1, b, kt, 0:HD], pst[:, HD : 2 * HD])

        # ---- phase 1: projections + rms norm (software-pipelined) ----
        specs = []
        for b in range(B):
            for g in range(T // GR):
                specs.append(("q", wq_sb, qn_sb, b, g))
                specs.append(("k", wk_sb, kn_sb, b, g))
                specs.append(("v", wv_sb, None, b, g))
        pending = None
        for kind, w_sb, dst, b, g in specs:
            ps = proj_psum(w_sb, b, g)
            if pending is not None:
                pending()
            if kind == "v":
                pending = (lambda ps=ps, b=b, g=g: v_post(ps, b, g))
            else:
                pending = (lambda ps=ps, dst=dst, b=b, g=g: rms_post(ps, dst, b, g))
        pending()

        # ---- phase 2: attention, phase 3: output projection ----
        def attention_batch(b, extra_work=None):
            def qk_tile(kt):
                k0 = b * T + kt * 128
                ps_s = pp.tile([128, 2 * JQ], dt.float32, tag="ps")
                nc.tensor.matmul(
                    ps_s[:, 0:JQ],
                    lhsT=kn_sb[0:64, k0 : k0 + 128],
                    rhs=qn_sb[0:64, q0 : q0 + JQ],
                    start=True,
                    stop=True,
                )
                nc.tensor.matmul(
                    ps_s[:, JQ : 2 * JQ],
                    lhsT=kn_sb[64:128, k0 : k0 + 128],
                    rhs=qn_sb[64:128, q0 : q0 + JQ],
                    start=True,
                    stop=True,
                )
                return ps_s

            for jq in range(NJQ):
                q0 = b * T + jq * JQ
                yA = pps.tile([HD + 1, JQ], dt.float32, tag="sm")
                yB = pps.tile([HD + 1, JQ], dt.float32, tag="sm")
                ps_cur = qk_tile(0)
                for kt in range(NKT):
                    se = se_pool.tile([128, 2 * JQ], dt.bfloat16)
                    nc.scalar.activation(
                        out=se[:],
                        in_=ps_cur[:],
                        func=AF.Exp,
                        bias=bias_sb[:, b * NKT + kt : b * NKT + kt + 1],
                        scale=SCALE,
                    )
                    if kt + 1 < NKT:
                        ps_cur = qk_tile(kt + 1)
                    nc.tensor.matmul(
                        yA[:],
                        lhsT=vext[:, 0, b, kt, :],
                        rhs=se[:, 0:JQ],
                        start=(kt == 0),
                        stop=(kt == NKT - 1),
                    )
                    nc.tensor.matmul(
                        yB[:],
                        lhsT=vext[:, 1, b, kt, :],
                        rhs=se[:, JQ : 2 * JQ],
                        start=(kt == 0),
                        stop=(kt == NKT - 1),
                    )
                # normalize: y / sum(exp); sums sit in row 64 of yA/yB
                rsv = rb2.tile([65, 2 * JQ], dt.float32, tag="rs")
                nc.vector.tensor_copy(rsv[64:65, 0:JQ], yA[HD : HD + 1, :])
                nc.vector.tensor_copy(rsv[64:65, JQ : 2 * JQ], yB[HD : HD + 1, :])
                rs0 = rb2.tile([1, 2 * JQ], dt.float32, tag="rs0")
                nc.sync.dma_start(out=rs0[:], in_=rsv[64:65, :])
                rr = rb2.tile([1, 2 * JQ], dt.float32, tag="rs0")
                nc.vector.reciprocal_approx_fast(out=rr[:], in_=rs0[:])
                rbA = rb2.tile([64, JQ], dt.float32, tag="rb")
                rbB0 = rb2.tile([64, JQ], dt.float32, tag="rb")
                nc.gpsimd.partition_broadcast(rbA[:], rr[0:1, 0:JQ])
                nc.gpsimd.partition_broadcast(rbB0[:], rr[0:1, JQ : 2 * JQ])
                nc.vector.tensor_mul(yn_sb[0:64, q0 : q0 + JQ], yA[0:64, :], rbA[:])
                ynB = work.tile([64, JQ], dt.bfloat16, tag="ynB")
                nc.vector.tensor_mul(ynB[:], yB[0:64, :], rbB0[:])
                nc.sync.dma_start(out=yn_sb[64:128, q0 : q0 + JQ], in_=ynB[:])
                if extra_work is not None:
                    extra_work(jq)

        def outproj_tile(b, dtile):
            ob = work.tile([128, T], dt.bfloat16, tag="ob")
            for ch in range(T // 512):
                ps_o = pps.tile([128, 512], dt.float32, tag="sm")
                nc.tensor.matmul(
                    ps_o[:],
                    lhsT=wo_sb[:, ts(dtile, 128)],
                    rhs=yn_sb[:, b * T + ch * 512 : b * T + (ch + 1) * 512],
                    start=True,
                    stop=True,
                )
                nc.vector.tensor_copy(ob[:, ts(ch, 512)], ps_o[:])
            nc.sync.dma_start(
                out=out_ext.ap()[:, dtile * BT + b * T : dtile * BT + (b + 1) * T],
                in_=ob[:],
            )

        def outproj_b0(jq):
            outproj_tile(0, 2 * jq)
            outproj_tile(0, 2 * jq + 1)

        attention_batch(0)
        attention_batch(1, extra_work=outproj_b0)
        for dtile in range(NCI):
            outproj_tile(1, dtile)

        if debug:
            dbg_specs = [
                ("dbg_qn", qn_sb, [128, BT], dt.bfloat16),
                ("dbg_kn", kn_sb, [128, BT], dt.bfloat16),
                ("dbg_yn", yn_sb, [128, BT], dt.bfloat16),
                ("dbg_vext", vext, [128, 2 * B * NKT * (HD + 1)], dt.bfloat16),
            ]
            for name, src, shape, dty in dbg_specs:
                ext = nc.dram_tensor(name, shape, dty, kind="ExternalOutput")
                flat = src[:]
                if len(flat.shape) > 2:
                    flat = flat.rearrange("p a b c d -> p (a b c d)")
                nc.sync.dma_start(out=ext.ap(), in_=flat)

    nc._dbg = {
        "qn": qn_sb.tensor.name,
        "kn": kn_sb.tensor.name,
        "vext": vext.tensor.name,
        "yn": yn_sb.tensor.name,
        "xt": xt_sb.tensor.name,
        "wq": wq_sb.tensor.name,
    }
    nc.compile()
    return nc


def _get_nc():
    if "nc" not in _CACHE:
        _CACHE["nc"] = _build_bass()
    return _CACHE["nc"]


def _tile_major(a, width):
    """[C, width] -> [128, NCI*width]: c-tile-major columns, partition = c%128."""
    return np.ascontiguousarray(
        a.reshape(NCI, 128, width).transpose(1, 0, 2).reshape(128, NCI * width)
    )


def _prep_in_maps(x, padding_mask, Wq, Wk, Wv, Wo):
    xf = np.ascontiguousarray(np.asarray(x, dtype=np.float32).reshape(BT, C))
    xt = _tile_major(np.ascontiguousarray(xf.T), BT).astype(BF16)
    mb = np.where(
        np.asarray(padding_mask).reshape(BT), np.float32(0.0), np.float32(NEG)
    ).astype(np.float32)
    bias = np.ascontiguousarray(mb.reshape(B * NKT, 128).T)

    in_maps = []
    for i in range(NCORES):
        sl = slice(i * DPC, (i + 1) * DPC)
        in_maps.append(
            {
                "xt": xt,
                "wq": _tile_major(np.ascontiguousarray(Wq[sl, :].T), DPC).astype(BF16),
                "wk": _tile_major(np.ascontiguousarray(Wk[sl, :].T), DPC).astype(BF16),
                "wv": _tile_major(np.ascontiguousarray(Wv[sl, :].T), DPC).astype(BF16),
                "wo": np.ascontiguousarray(Wo[:, sl].T).astype(BF16),
                "bias": bias,
            }
        )
    return in_maps


def _assemble(results):
    total = np.zeros((NCI, 128, BT), dtype=np.float32)
    for r in results:
        total += (
            r["out"].reshape(128, NCI, BT).transpose(1, 0, 2).astype(np.float32)
        )
    return np.ascontiguousarray(total.reshape(C, BT).T).reshape(B, T, C)


def kernel(x, padding_mask, Wq, Wk, Wv, Wo):
    from concourse.bass_utils import run_bass_kernel_spmd

    nc = _get_nc()
    in_maps = _prep_in_maps(x, padding_mask, Wq, Wk, Wv, Wo)
    res = run_bass_kernel_spmd(nc, in_maps, core_ids=list(range(NCORES)))
    return _assemble(res.results)


# revision 29
# speedup vs baseline: 1.6672x; 1.0042x over previous
"""Tensor-parallel multi-head attention kernel for 8 Trainium2 NeuronCores.

Sharding: tensor-parallel over heads. Each core owns 2 of the 16 heads
(a 128-dim slice of the projections). Wq/Wk/Wv are sharded column-wise
(output dim), Wo row-wise (input dim). Each core computes a full-shape
partial of the output projection; the host sums the 8 partials (the
"all-reduce") and transposes back. No device collectives are needed.

All device matmuls run in bf16 with f32 PSUM accumulation. The emission
order software-pipelines three streams so no engine idles long enough to
lose the HAM clock boost:
  head:  projections for batch 0
  attn(b0): exp-bound loop, with batch-1 projection granules as PE filler
  attn(b1): with batch-0 output-projection tiles as PE filler
  tail:  batch-1 output projection
"""

import sys

if "/opt/trn_rl_repo" not in sys.path:
    sys.path.insert(0, "/opt/trn_rl_repo")

import numpy as np
import ml_dtypes

BF16 = ml_dtypes.bfloat16

B, T, C = 2, 2048, 1024
H, HD = 16, 64
BT = B * T            # 4096 tokens total
NCORES = 8
DPC = C // NCORES     # 128 projection dims per core (2 heads x 64)
NKT = T // 128        # 16 k-tiles of 128 tokens per batch
NCI = C // 128        # 8 contraction tiles for the projections
SCALE = 1.0 / 8.0     # 1/sqrt(HD)
EPS = float(np.finfo(np.float32).eps)
JQ = 512              # query-chunk width in the attention loop
NJQ = T // JQ         # 4 query chunks per batch
GR = 512              # projection granule (tokens)
NG = T // GR          # granules per batch per tensor
NEG = -30000.0        # additive mask bias for padded keys

_CACHE = {}


def _build_bass():
    import os
    import concourse.bass as bass
    from concourse import bacc, mybir, tile
    from concourse.masks import make_identity
    from contextlib import ExitStack

    dt = mybir.dt
    AF = mybir.ActivationFunctionType
    ts = bass.ts

    debug = bool(int(os.environ.get("BASS_ATTN_DEBUG", "0")))

    nc = bacc.Bacc("TRN2", target_bir_lowering=False, debug=False)

    xt_ext = nc.dram_tensor("xt", [128, NCI * BT], dt.bfloat16, kind="ExternalInput")
    wq_ext = nc.dram_tensor("wq", [128, NCI * DPC], dt.bfloat16, kind="ExternalInput")
    wk_ext = nc.dram_tensor("wk", [128, NCI * DPC], dt.bfloat16, kind="ExternalInput")
    wv_ext = nc.dram_tensor("wv", [128, NCI * DPC], dt.bfloat16, kind="ExternalInput")
    wo_ext = nc.dram_tensor("wo", [DPC, C], dt.bfloat16, kind="ExternalInput")
    bias_ext = nc.dram_tensor("bias", [128, B * NKT], dt.float32, kind="ExternalInput")
    out_ext = nc.dram_tensor("out", [128, NCI * BT], dt.bfloat16, kind="ExternalOutput")

    with ExitStack() as ctx:
        tc = ctx.enter_context(tile.TileContext(nc))
        singles = ctx.enter_context(tc.tile_pool(name="singles", bufs=1))
        work = ctx.enter_context(tc.tile_pool(name="work", bufs=3))
        se_pool = ctx.enter_context(tc.tile_pool(name="se", bufs=3))
        rbp = ctx.enter_context(tc.tile_pool(name="rb", bufs=3))
        rb2 = ctx.enter_context(tc.tile_pool(name="rb2", bufs=2))
        pp = ctx.enter_context(tc.tile_pool(name="pp", bufs=2, space="PSUM"))
        pps = ctx.enter_context(tc.tile_pool(name="pps", bufs=4, space="PSUM"))

        # ---- persistent SBUF state ----
        xt_sb = singles.tile([128, NCI * BT], dt.bfloat16)     # xT, ci-major
        wq_sb = singles.tile([128, NCI * DPC], dt.bfloat16)
        wk_sb = singles.tile([128, NCI * DPC], dt.bfloat16)
        wv_sb = singles.tile([128, NCI * DPC], dt.bfloat16)
        wo_sb = singles.tile([128, C], dt.bfloat16)
        bias_sb = singles.tile([128, B * NKT], dt.float32)
        qn_sb = singles.tile([128, BT], dt.bfloat16)           # rms-normed qT
        kn_sb = singles.tile([128, BT], dt.bfloat16)           # rms-normed kT
        # v tiles per (head, batch, k-tile): [v | ones] -> M=65 PV matmuls
        vext = singles.tile([128, 2, B, NKT, HD + 1], dt.bfloat16)
        yn_sb = singles.tile([128, BT], dt.bfloat16)           # normalized yT
        onesAB = singles.tile([128, 33], dt.bfloat16)  # col0: headA, col32: headB
        ident = singles.tile([128, 128], dt.bfloat16)
        eps_sb = singles.tile([128, 1], dt.float32)
        zero_sb = singles.tile([128, 1], dt.float32)

        nc.sync.dma_start(out=xt_sb[:], in_=xt_ext.ap())
        nc.sync.dma_start(out=wq_sb[:], in_=wq_ext.ap())
        nc.sync.dma_start(out=wk_sb[:], in_=wk_ext.ap())
        nc.sync.dma_start(out=wv_sb[:], in_=wv_ext.ap())
        nc.sync.dma_start(out=wo_sb[:], in_=wo_ext.ap())
        nc.sync.dma_start(out=bias_sb[:], in_=bias_ext.ap())

        nc.gpsimd.memset(eps_sb[:], EPS)
        nc.gpsimd.memset(zero_sb[:], 0.0)
        nc.gpsimd.memset(onesAB[:], 0.0)
        nc.gpsimd.memset(onesAB[0:64, 0:1], 1.0)
        nc.gpsimd.memset(onesAB[64:128, 32:33], 1.0)
        nc.gpsimd.memset(vext[:, :, :, :, HD : HD + 1], 1.0)
        make_identity(nc, ident[:])

        def proj_psum(w_sb, t0):
            """Project GR tokens starting at t0 -> psum [128, GR] (f32)."""
            ps = pps.tile([128, GR], dt.float32, tag="sm")
            for ci in range(NCI):
                nc.tensor.matmul(
                    ps[:],
                    lhsT=w_sb[:, ts(ci, DPC)],
                    rhs=xt_sb[:, ci * BT + t0 : ci * BT + t0 + GR],
                    start=(ci == 0),
                    stop=(ci == NCI - 1),
                )
            return ps

        def rms_granule(w_sb, dst_sb, t0):
            """Project + rms-normalize GR tokens per head, store bf16 into dst."""
            ps = proj_psum(w_sb, t0)
            q2 = work.tile([128, GR], dt.bfloat16, tag="sb512")
            nc.scalar.activation(out=q2[:], in_=ps[:], func=AF.Square)
            ssq = pps.tile([33, GR], dt.float32, tag="sm")
            nc.tensor.matmul(ssq[:], lhsT=onesAB[:], rhs=q2[:], start=True, stop=True)
            lnt = rbp.tile([33, GR], dt.float32, tag="lnt")
            # rows 0 / 32 hold per-head sum(x^2); ln(ms + eps)
            nc.scalar.activation(
                out=lnt[:], in_=ssq[:], func=AF.Ln, bias=eps_sb[0:33, :], scale=1.0 / HD
            )
            rinv = rbp.tile([33, GR], dt.float32, tag="lnt")
            # rinv = exp(-0.5 ln(ms + eps)) = rsqrt(ms + eps)
            nc.scalar.activation(
                out=rinv[:], in_=lnt[:], func=AF.Exp, bias=zero_sb[0:33, :], scale=-0.5
            )
            # gather head rows {0, 32} to partition 0, broadcast per head
            rsb = rbp.tile([1, 2 * GR], dt.float32, tag="rsb")
            pstep = rinv[:].ap[0][0]
            src = bass.AP(
                tensor=rinv.tensor,
                offset=rinv[:].offset,
                ap=[[32 * pstep, 2]] + rinv[0:1, :].ap[1:],
            )
            nc.sync.dma_start(out=rsb[:].rearrange("p (a b) -> p a b", a=2), in_=src)
            rbc = rbp.tile([128, GR], dt.float32, tag="rbc")
            rbB0 = rbp.tile([64, GR], dt.float32, tag="rbB")
            nc.gpsimd.partition_broadcast(rbc[0:64, :], rsb[0:1, 0:GR])
            nc.gpsimd.partition_broadcast(rbB0[:], rsb[0:1, GR : 2 * GR])
            nc.sync.dma_start(out=rbc[64:128, :], in_=rbB0[:])
            nc.vector.tensor_mul(dst_sb[:, t0 : t0 + GR], ps[:], rbc[:])

        def v_granule(b, g):
            """Project GR tokens of v, transpose 128-blocks into vext."""
            t0 = b * T + g * GR
            ps = proj_psum(wv_sb, t0)
            vt = work.tile([128, GR], dt.bfloat16, tag="sb512")
            nc.scalar.copy(vt[:], ps[:])
            for j in range(GR // 128):
                pst = pps.tile([128, 128], dt.bfloat16, tag="sm")
                nc.tensor.transpose(pst[:], vt[:, ts(j, 128)], ident[:])
                kt = g * (GR // 128) + j
                nc.vector.tensor_copy(vext[:, 0, b, kt, 0:HD], pst[:, 0:HD])
                nc.vector.tensor_copy(vext[:, 1, b, kt, 0:HD], pst[:, HD : 2 * HD])

        def proj_thunks(b):
            th = []
            for g in range(NG):
                th.append(lambda b=b, g=g: rms_granule(wq_sb, qn_sb, b * T + g * GR))
            for g in range(NG):
                th.append(lambda b=b, g=g: rms_granule(wk_sb, kn_sb, b * T + g * GR))
            for g in range(NG):
                th.append(lambda b=b, g=g: v_granule(b, g))
            return th

        def outproj_tile(b, dtile):
            ob = work.tile([128, T], dt.bfloat16, tag="ob")
            for ch in range(T // 512):
                ps_o = pps.tile([128, 512], dt.float32, tag="sm")
                nc.tensor.matmul(
                    ps_o[:],
                    lhsT=wo_sb[:, ts(dtile, 128)],
                    rhs=yn_sb[:, b * T + ch * 512 : b * T + (ch + 1) * 512],
                    start=True,
                    stop=True,
                )
                nc.vector.tensor_copy(ob[:, ts(ch, 512)], ps_o[:])
            nc.sync.dma_start(
                out=out_ext.ap()[:, dtile * BT + b * T : dtile * BT + (b + 1) * T],
                in_=ob[:],
            )

        def attention_batch(b, thunks=()):
            """exp-bound attention loop; `thunks` are emitted as PE filler."""
            tq = list(thunks)
            slots = {}
            if tq:
                step = (NJQ * NKT) // len(tq)
                for i, th in enumerate(tq):
                    it = min(i * step + step // 2, NJQ * NKT - 1)
                    slots.setdefault(it, []).append(th)

            def qk_tile(q0, kt):
                k0 = b * T + kt * 128
                ps_s = pp.tile([128, 2 * JQ], dt.float32, tag="ps")
                nc.tensor.matmul(
                    ps_s[:, 0:JQ],
                    lhsT=kn_sb[0:64, k0 : k0 + 128],
                    rhs=qn_sb[0:64, q0 : q0 + JQ],
                    start=True,
                    stop=True,
                )
                nc.tensor.matmul(
                    ps_s[:, JQ : 2 * JQ],
                    lhsT=kn_sb[64:128, k0 : k0 + 128],
                    rhs=qn_sb[64:128, q0 : q0 + JQ],
                    start=True,
                    stop=True,
                )
                return ps_s

            for jq in range(NJQ):
                q0 = b * T + jq * JQ
                yA = pps.tile([HD + 1, JQ], dt.float32, tag="sm")
                yB = pps.tile([HD + 1, JQ], dt.float32, tag="sm")
                if jq == 0:
                    ps_cur = qk_tile(q0, 0)
                for kt in range(NKT):
                    se = se_pool.tile([128, 2 * JQ], dt.bfloat16)
                    nc.scalar.activation(
                        out=se[:],
                        in_=ps_cur[:],
                        func=AF.Exp,
                        bias=bias_sb[:, b * NKT + kt : b * NKT + kt + 1],
                        scale=SCALE,
                    )
                    if kt + 1 < NKT:
                        ps_cur = qk_tile(q0, kt + 1)
                    elif jq + 1 < NJQ:
                        ps_cur = qk_tile(b * T + (jq + 1) * JQ, 0)
                    nc.tensor.matmul(
                        yA[:],
                        lhsT=vext[:, 0, b, kt, :],
                        rhs=se[:, 0:JQ],
                        start=(kt == 0),
                        stop=(kt == NKT - 1),
                    )
                    nc.tensor.matmul(
                        yB[:],
                        lhsT=vext[:, 1, b, kt, :],
                        rhs=se[:, JQ : 2 * JQ],
                        start=(kt == 0),
                        stop=(kt == NKT - 1),
                    )
                    for th in slots.get(jq * NKT + kt, ()):
                        th()
                # normalize: y / sum(exp); sums sit in row 64 of yA/yB
                rsv = rb2.tile([65, 2 * JQ], dt.float32, tag="rs")
                nc.vector.tensor_copy(rsv[64:65, 0:JQ], yA[HD : HD + 1, :])
                nc.vector.tensor_copy(rsv[64:65, JQ : 2 * JQ], yB[HD : HD + 1, :])
                rs0 = rb2.tile([1, 2 * JQ], dt.float32, tag="rs0")
                nc.sync.dma_start(out=rs0[:], in_=rsv[64:65, :])
                rr = rb2.tile([1, 2 * JQ], dt.float32, tag="rs0")
                nc.vector.reciprocal_approx_fast(out=rr[:], in_=rs0[:])
                rbA = rb2.tile([64, JQ], dt.float32, tag="rb")
                rbB0 = rb2.tile([64, JQ], dt.float32, tag="rb")
                nc.gpsimd.partition_broadcast(rbA[:], rr[0:1, 0:JQ])
                nc.gpsimd.partition_broadcast(rbB0[:], rr[0:1, JQ : 2 * JQ])
                nc.vector.tensor_mul(yn_sb[0:64, q0 : q0 + JQ], yA[0:64, :], rbA[:])
                ynB = work.tile([64, JQ], dt.bfloat16, tag="ynB")
                nc.vector.tensor_mul(ynB[:], yB[0:64, :], rbB0[:])
                nc.sync.dma_start(out=yn_sb[64:128, q0 : q0 + JQ], in_=ynB[:])

        # ---- emission schedule ----
        for th in proj_thunks(0):
            th()
        attention_batch(0, thunks=proj_thunks(1))
        attention_batch(
            1, thunks=[lambda d=d: outproj_tile(0, d) for d in range(NCI)]
        )
        for dtile in range(NCI):
            outproj_tile(1, dtile)

        if debug:
            dbg_specs = [
                ("dbg_qn", qn_sb, [128, BT], dt.bfloat16),
                ("dbg_kn", kn_sb, [128, BT], dt.bfloat16),
                ("dbg_yn", yn_sb, [128, BT], dt.bfloat16),
                ("dbg_vext", vext, [128, 2 * B * NKT * (HD + 1)], dt.bfloat16),
            ]
            for name, srct, shape, dty in dbg_specs:
                extd = nc.dram_tensor(name, shape, dty, kind="ExternalOutput")
                flat = srct[:]
                if len(flat.shape) > 2:
                    flat = flat.rearrange("p a b c d -> p (a b c d)")
                nc.sync.dma_start(out=extd.ap(), in_=flat)

    nc._dbg = {
        "qn": qn_sb.tensor.name,
        "kn": kn_sb.tensor.name,
        "vext": vext.tensor.name,
        "yn": yn_sb.tensor.name,
    }
    nc.compile()
    return nc


def _get_nc():
    if "nc" not in _CACHE:
        _CACHE["nc"] = _build_bass()
    return _CACHE["nc"]


def _tile_major(a, width):
    """[C, width] -> [128, NCI*width]: c-tile-major columns, partition = c%128."""
    return np.ascontiguousarray(
        a.reshape(NCI, 128, width).transpose(1, 0, 2).reshape(128, NCI * width)
    )


def _prep_in_maps(x, padding_mask, Wq, Wk, Wv, Wo):
    xf = np.ascontiguousarray(np.asarray(x, dtype=np.float32).reshape(BT, C))
    xt = _tile_major(np.ascontiguousarray(xf.T), BT).astype(BF16)
    mb = np.where(
        np.asarray(padding_mask).reshape(BT), np.float32(0.0), np.float32(NEG)
    ).astype(np.float32)
    bias = np.ascontiguousarray(mb.reshape(B * NKT, 128).T)

    in_maps = []
    for i in range(NCORES):
        sl = slice(i * DPC, (i + 1) * DPC)
        in_maps.append(
            {
                "xt": xt,
                "wq": _tile_major(np.ascontiguousarray(Wq[sl, :].T), DPC).astype(BF16),
                "wk": _tile_major(np.ascontiguousarray(Wk[sl, :].T), DPC).astype(BF16),
                "wv": _tile_major(np.ascontiguousarray(Wv[sl, :].T), DPC).astype(BF16),
                "wo": np.ascontiguousarray(Wo[:, sl].T).astype(BF16),
                "bias": bias,
            }
        )
    return in_maps


def _assemble(results):
    total = np.zeros((NCI, 128, BT), dtype=np.float32)
    for r in results:
        total += (
            r["out"].reshape(128, NCI, BT).transpose(1, 0, 2).astype(np.float32)
        )
    return np.ascontiguousarray(total.reshape(C, BT).T).reshape(B, T, C)


def kernel(x, padding_mask, Wq, Wk, Wv, Wo):
    from concourse.bass_utils import run_bass_kernel_spmd

    nc = _get_nc()
    in_maps = _prep_in_maps(x, padding_mask, Wq, Wk, Wv, Wo)
    res = run_bass_kernel_spmd(nc, in_maps, core_ids=list(range(NCORES)))
    return _assemble(res.results)


# revision 30
# speedup vs baseline: 1.9049x; 1.1426x over previous
"""Tensor-parallel multi-head attention kernel for 8 Trainium2 NeuronCores.

Sharding: tensor-parallel over heads. Each core owns 2 of the 16 heads
(a 128-dim slice of the projections). Wq/Wk/Wv are sharded column-wise
(output dim), Wo row-wise (input dim). Each core computes a full-shape
partial of the output projection; the host sums the 8 partials (the
"all-reduce") and transposes back. No device collectives are needed.

All device matmuls run in bf16 with f32 PSUM accumulation. The emission
order software-pipelines three streams so no engine idles long enough to
lose the HAM clock boost:
  head:  projections for batch 0
  attn(b0): exp-bound loop, with batch-1 projection granules as PE filler
  attn(b1): with batch-0 output-projection tiles as PE filler
  tail:  batch-1 output projection
"""

import sys

if "/opt/trn_rl_repo" not in sys.path:
    sys.path.insert(0, "/opt/trn_rl_repo")

import numpy as np
import ml_dtypes

BF16 = ml_dtypes.bfloat16

B, T, C = 2, 2048, 1024
H, HD = 16, 64
BT = B * T            # 4096 tokens total
NCORES = 8
DPC = C // NCORES     # 128 projection dims per core (2 heads x 64)
NKT = T // 128        # 16 k-tiles of 128 tokens per batch
NCI = C // 128        # 8 contraction tiles for the projections
SCALE = 1.0 / 8.0     # 1/sqrt(HD)
EPS = float(np.finfo(np.float32).eps)
JQ = 512              # query-chunk width in the attention loop
NJQ = T // JQ         # 4 query chunks per batch
GR = 512              # projection granule (tokens)
NG = T // GR          # granules per batch per tensor
NEG = -30000.0        # additive mask bias for padded keys

_CACHE = {}


def _build_bass():
    import os
    import concourse.bass as bass
    from concourse import bacc, mybir, tile
    from concourse.masks import make_identity
    from contextlib import ExitStack

    dt = mybir.dt
    AF = mybir.ActivationFunctionType
    ts = bass.ts

    debug = bool(int(os.environ.get("BASS_ATTN_DEBUG", "0")))

    # Force all activations onto natural_log_exp_and_others (it contains
    # exp/ln/square/copy), so the kernel needs exactly one ACT table load
    # instead of thrashing between exp_and_others and natural_log sets.
    from concourse import bacc as _bacc_mod, hw_specs as _hw

    _orig_tables = _hw.get_activation_tables

    def _only_nl_exp(arch):
        t = _orig_tables(arch)
        return {
            name: (fns if name == "natural_log_exp_and_others" else set())
            for name, fns in t.items()
        }

    _bacc_mod.get_activation_tables = _only_nl_exp

    nc = bacc.Bacc("TRN2", target_bir_lowering=False, debug=False)

    xt_ext = nc.dram_tensor("xt", [128, NCI * BT], dt.bfloat16, kind="ExternalInput")
    wq_ext = nc.dram_tensor("wq", [128, NCI * DPC], dt.bfloat16, kind="ExternalInput")
    wk_ext = nc.dram_tensor("wk", [128, NCI * DPC], dt.bfloat16, kind="ExternalInput")
    wv_ext = nc.dram_tensor("wv", [128, NCI * DPC], dt.bfloat16, kind="ExternalInput")
    wo_ext = nc.dram_tensor("wo", [DPC, C], dt.bfloat16, kind="ExternalInput")
    bias_ext = nc.dram_tensor("bias", [128, B * NKT], dt.float32, kind="ExternalInput")
    out_ext = nc.dram_tensor("out", [128, NCI * BT], dt.bfloat16, kind="ExternalOutput")

    with ExitStack() as ctx:
        tc = ctx.enter_context(tile.TileContext(nc))
        singles = ctx.enter_context(tc.tile_pool(name="singles", bufs=1))
        work = ctx.enter_context(tc.tile_pool(name="work", bufs=3))
        se_pool = ctx.enter_context(tc.tile_pool(name="se", bufs=3))
        rbp = ctx.enter_context(tc.tile_pool(name="rb", bufs=3))
        rb2 = ctx.enter_context(tc.tile_pool(name="rb2", bufs=2))
        pp = ctx.enter_context(tc.tile_pool(name="pp", bufs=2, space="PSUM"))
        pps = ctx.enter_context(tc.tile_pool(name="pps", bufs=4, space="PSUM"))

        # ---- persistent SBUF state ----
        xt_sb = singles.tile([128, NCI * BT], dt.bfloat16)     # xT, ci-major
        wq_sb = singles.tile([128, NCI * DPC], dt.bfloat16)
        wk_sb = singles.tile([128, NCI * DPC], dt.bfloat16)
        wv_sb = singles.tile([128, NCI * DPC], dt.bfloat16)
        wo_sb = singles.tile([128, C], dt.bfloat16)
        bias_sb = singles.tile([128, B * NKT], dt.float32)
        qn_sb = singles.tile([128, BT], dt.bfloat16)           # rms-normed qT
        kn_sb = singles.tile([128, BT], dt.bfloat16)           # rms-normed kT
        # v tiles per (head, batch, k-tile): [v | ones] -> M=65 PV matmuls
        vext = singles.tile([128, 2, B, NKT, HD + 1], dt.bfloat16)
        yn_sb = singles.tile([128, BT], dt.bfloat16)           # normalized yT
        onesAB = singles.tile([128, 33], dt.bfloat16)  # col0: headA, col32: headB
        ident = singles.tile([128, 128], dt.bfloat16)
        eps_sb = singles.tile([128, 1], dt.float32)
        zero_sb = singles.tile([128, 1], dt.float32)

        nc.sync.dma_start(out=xt_sb[:], in_=xt_ext.ap())
        nc.sync.dma_start(out=wq_sb[:], in_=wq_ext.ap())
        nc.sync.dma_start(out=wk_sb[:], in_=wk_ext.ap())
        nc.sync.dma_start(out=wv_sb[:], in_=wv_ext.ap())
        nc.sync.dma_start(out=wo_sb[:], in_=wo_ext.ap())
        nc.sync.dma_start(out=bias_sb[:], in_=bias_ext.ap())

        nc.gpsimd.memset(eps_sb[:], EPS)
        nc.gpsimd.memset(zero_sb[:], 0.0)
        nc.gpsimd.memset(onesAB[:], 0.0)
        nc.gpsimd.memset(onesAB[0:64, 0:1], 1.0)
        nc.gpsimd.memset(onesAB[64:128, 32:33], 1.0)
        nc.gpsimd.memset(vext[:, :, :, :, HD : HD + 1], 1.0)
        make_identity(nc, ident[:])

        def proj_psum(w_sb, t0):
            """Project GR tokens starting at t0 -> psum [128, GR] (f32)."""
            ps = pps.tile([128, GR], dt.float32, tag="sm")
            for ci in range(NCI):
                nc.tensor.matmul(
                    ps[:],
                    lhsT=w_sb[:, ts(ci, DPC)],
                    rhs=xt_sb[:, ci * BT + t0 : ci * BT + t0 + GR],
                    start=(ci == 0),
                    stop=(ci == NCI - 1),
                )
            return ps

        def rms_granule(w_sb, dst_sb, t0):
            """Project + rms-normalize GR tokens per head, store bf16 into dst."""
            ps = proj_psum(w_sb, t0)
            q2 = work.tile([128, GR], dt.bfloat16, tag="sb512")
            nc.scalar.activation(out=q2[:], in_=ps[:], func=AF.Square)
            ssq = pps.tile([33, GR], dt.float32, tag="sm")
            nc.tensor.matmul(ssq[:], lhsT=onesAB[:], rhs=q2[:], start=True, stop=True)
            lnt = rbp.tile([33, GR], dt.float32, tag="lnt")
            # rows 0 / 32 hold per-head sum(x^2); ln(ms + eps)
            nc.scalar.activation(
                out=lnt[:], in_=ssq[:], func=AF.Ln, bias=eps_sb[0:33, :], scale=1.0 / HD
            )
            rinv = rbp.tile([33, GR], dt.float32, tag="lnt")
            # rinv = exp(-0.5 ln(ms + eps)) = rsqrt(ms + eps)
            nc.scalar.activation(
                out=rinv[:], in_=lnt[:], func=AF.Exp, bias=zero_sb[0:33, :], scale=-0.5
            )
            # gather head rows {0, 32} to partition 0, broadcast per head
            rsb = rbp.tile([1, 2 * GR], dt.float32, tag="rsb")
            pstep = rinv[:].ap[0][0]
            src = bass.AP(
                tensor=rinv.tensor,
                offset=rinv[:].offset,
                ap=[[32 * pstep, 2]] + rinv[0:1, :].ap[1:],
            )
            nc.sync.dma_start(out=rsb[:].rearrange("p (a b) -> p a b", a=2), in_=src)
            rbc = rbp.tile([128, GR], dt.float32, tag="rbc")
            rbB0 = rbp.tile([64, GR], dt.float32, tag="rbB")
            nc.gpsimd.partition_broadcast(rbc[0:64, :], rsb[0:1, 0:GR])
            nc.gpsimd.partition_broadcast(rbB0[:], rsb[0:1, GR : 2 * GR])
            nc.sync.dma_start(out=rbc[64:128, :], in_=rbB0[:])
            nc.vector.tensor_mul(dst_sb[:, t0 : t0 + GR], ps[:], rbc[:])

        def v_granule(b, g):
            """Project GR tokens of v, transpose 128-blocks into vext."""
            t0 = b * T + g * GR
            ps = proj_psum(wv_sb, t0)
            vt = work.tile([128, GR], dt.bfloat16, tag="sb512")
            nc.vector.tensor_copy(vt[:], ps[:])
            for j in range(GR // 128):
                pst = pps.tile([128, 128], dt.bfloat16, tag="sm")
                nc.tensor.transpose(pst[:], vt[:, ts(j, 128)], ident[:])
                kt = g * (GR // 128) + j
                nc.vector.tensor_copy(vext[:, 0, b, kt, 0:HD], pst[:, 0:HD])
                nc.vector.tensor_copy(vext[:, 1, b, kt, 0:HD], pst[:, HD : 2 * HD])

        def proj_thunks(b):
            th = []
            for g in range(NG):
                th.append(lambda b=b, g=g: rms_granule(wq_sb, qn_sb, b * T + g * GR))
            for g in range(NG):
                th.append(lambda b=b, g=g: rms_granule(wk_sb, kn_sb, b * T + g * GR))
            for g in range(NG):
                th.append(lambda b=b, g=g: v_granule(b, g))
            return th

        def outproj_tile(b, dtile):
            ob = work.tile([128, T], dt.bfloat16, tag="ob")
            for ch in range(T // 512):
                ps_o = pps.tile([128, 512], dt.float32, tag="sm")
                nc.tensor.matmul(
                    ps_o[:],
                    lhsT=wo_sb[:, ts(dtile, 128)],
                    rhs=yn_sb[:, b * T + ch * 512 : b * T + (ch + 1) * 512],
                    start=True,
                    stop=True,
                )
                nc.vector.tensor_copy(ob[:, ts(ch, 512)], ps_o[:])
            nc.sync.dma_start(
                out=out_ext.ap()[:, dtile * BT + b * T : dtile * BT + (b + 1) * T],
                in_=ob[:],
            )

        def attention_batch(b, thunks=()):
            """exp-bound attention loop; `thunks` are emitted as PE filler."""
            tq = list(thunks)
            slots = {}
            if tq:
                step = (NJQ * NKT) // len(tq)
                for i, th in enumerate(tq):
                    it = min(i * step + step // 2, NJQ * NKT - 1)
                    slots.setdefault(it, []).append(th)

            def qk_tile(q0, kt):
                k0 = b * T + kt * 128
                ps_s = pp.tile([128, 2 * JQ], dt.float32, tag="ps")
                nc.tensor.matmul(
                    ps_s[:, 0:JQ],
                    lhsT=kn_sb[0:64, k0 : k0 + 128],
                    rhs=qn_sb[0:64, q0 : q0 + JQ],
                    start=True,
                    stop=True,
                )
                nc.tensor.matmul(
                    ps_s[:, JQ : 2 * JQ],
                    lhsT=kn_sb[64:128, k0 : k0 + 128],
                    rhs=qn_sb[64:128, q0 : q0 + JQ],
                    start=True,
                    stop=True,
                )
                return ps_s

            for jq in range(NJQ):
                q0 = b * T + jq * JQ
                yA = pps.tile([HD + 1, JQ], dt.float32, tag="sm")
                yB = pps.tile([HD + 1, JQ], dt.float32, tag="sm")
                if jq == 0:
                    ps_cur = qk_tile(q0, 0)
                for kt in range(NKT):
                    se = se_pool.tile([128, 2 * JQ], dt.bfloat16)
                    nc.scalar.activation(
                        out=se[:],
                        in_=ps_cur[:],
                        func=AF.Exp,
                        bias=bias_sb[:, b * NKT + kt : b * NKT + kt + 1],
                        scale=SCALE,
                    )
                    if kt + 1 < NKT:
                        ps_cur = qk_tile(q0, kt + 1)
                    elif jq + 1 < NJQ:
                        ps_cur = qk_tile(b * T + (jq + 1) * JQ, 0)
                    nc.tensor.matmul(
                        yA[:],
                        lhsT=vext[:, 0, b, kt, :],
                        rhs=se[:, 0:JQ],
                        start=(kt == 0),
                        stop=(kt == NKT - 1),
                    )
                    nc.tensor.matmul(
                        yB[:],
                        lhsT=vext[:, 1, b, kt, :],
                        rhs=se[:, JQ : 2 * JQ],
                        start=(kt == 0),
                        stop=(kt == NKT - 1),
                    )
                    for th in slots.get(jq * NKT + kt, ()):
                        th()
                # normalize: y / sum(exp); sums sit in row 64 of yA/yB
                rsv = rb2.tile([65, 2 * JQ], dt.float32, tag="rs")
                nc.vector.tensor_copy(rsv[64:65, 0:JQ], yA[HD : HD + 1, :])
                nc.vector.tensor_copy(rsv[64:65, JQ : 2 * JQ], yB[HD : HD + 1, :])
                rs0 = rb2.tile([1, 2 * JQ], dt.float32, tag="rs0")
                nc.sync.dma_start(out=rs0[:], in_=rsv[64:65, :])
                rr = rb2.tile([1, 2 * JQ], dt.float32, tag="rs0")
                nc.vector.reciprocal_approx_fast(out=rr[:], in_=rs0[:])
                rbA = rb2.tile([64, JQ], dt.float32, tag="rb")
                rbB0 = rb2.tile([64, JQ], dt.float32, tag="rb")
                nc.gpsimd.partition_broadcast(rbA[:], rr[0:1, 0:JQ])
                nc.gpsimd.partition_broadcast(rbB0[:], rr[0:1, JQ : 2 * JQ])
                nc.vector.tensor_mul(yn_sb[0:64, q0 : q0 + JQ], yA[0:64, :], rbA[:])
                ynB = work.tile([64, JQ], dt.bfloat16, tag="ynB")
                nc.vector.tensor_mul(ynB[:], yB[0:64, :], rbB0[:])
                nc.sync.dma_start(out=yn_sb[64:128, q0 : q0 + JQ], in_=ynB[:])

        # ---- emission schedule ----
        for th in proj_thunks(0):
            th()
        attention_batch(0, thunks=proj_thunks(1))
        attention_batch(
            1, thunks=[lambda d=d: outproj_tile(0, d) for d in range(NCI)]
        )
        for dtile in range(NCI):
            outproj_tile(1, dtile)

        if debug:
            dbg_specs = [
                ("dbg_qn", qn_sb, [128, BT], dt.bfloat16),
                ("dbg_kn", kn_sb, [128, BT], dt.bfloat16),
                ("dbg_yn", yn_sb, [128, BT], dt.bfloat16),
                ("dbg_vext", vext, [128, 2 * B * NKT * (HD + 1)], dt.bfloat16),
            ]
            for name, srct, shape, dty in dbg_specs:
                extd = nc.dram_tensor(name, shape, dty, kind="ExternalOutput")
                flat = srct[:]
                if len(flat.shape) > 2:
                    flat = flat.rearrange("p a b c d -> p (a b c d)")
                nc.sync.dma_start(out=extd.ap(), in_=flat)

    nc._dbg = {
        "qn": qn_sb.tensor.name,
        "kn": kn_sb.tensor.name,
        "vext": vext.tensor.name,
        "yn": yn_sb.tensor.name,
    }
    nc.compile()
    _bacc_mod.get_activation_tables = _orig_tables
    return nc


def _get_nc():
    if "nc" not in _CACHE:
        _CACHE["nc"] = _build_bass()
    return _CACHE["nc"]


def _tile_major(a, width):
    """[C, width] -> [128, NCI*width]: c-tile-major columns, partition = c%128."""
    return np.ascontiguousarray(
        a.reshape(NCI, 128, width).transpose(1, 0, 2).reshape(128, NCI * width)
    )


def _prep_in_maps(x, padding_mask, Wq, Wk, Wv, Wo):
    xf = np.ascontiguousarray(np.asarray(x, dtype=np.float32).reshape(BT, C))
    xt = _tile_major(np.ascontiguousarray(xf.T), BT).astype(BF16)
    mb = np.where(
        np.asarray(padding_mask).reshape(BT), np.float32(0.0), np.float32(NEG)
    ).astype(np.float32)
    bias = np.ascontiguousarray(mb.reshape(B * NKT, 128).T)

    in_maps = []
    for i in range(NCORES):
        sl = slice(i * DPC, (i + 1) * DPC)
        in_maps.append(
            {
                "xt": xt,
                "wq": _tile_major(np.ascontiguousarray(Wq[sl, :].T), DPC).astype(BF16),
                "wk": _tile_major(np.ascontiguousarray(Wk[sl, :].T), DPC).astype(BF16),
                "wv": _tile_major(np.ascontiguousarray(Wv[sl, :].T), DPC).astype(BF16),
                "wo": np.ascontiguousarray(Wo[:, sl].T).astype(BF16),
                "bias": bias,
            }
        )
    return in_maps


def _assemble(results):
    total = np.zeros((NCI, 128, BT), dtype=np.float32)
    for r in results:
        total += (
            r["out"].reshape(128, NCI, BT).transpose(1, 0, 2).astype(np.float32)
        )
    return np.ascontiguousarray(total.reshape(C, BT).T).reshape(B, T, C)


def kernel(x, padding_mask, Wq, Wk, Wv, Wo):
    from concourse.bass_utils import run_bass_kernel_spmd

    nc = _get_nc()
    in_maps = _prep_in_maps(x, padding_mask, Wq, Wk, Wv, Wo)
    res = run_bass_kernel_spmd(nc, in_maps, core_ids=list(range(NCORES)))
    return _assemble(res.results)
